# revision 1
# baseline (speedup 1.0000x reference)
"""Causal self-attention Trainium2 kernel (B=256, T=256, C=384, 8 heads x 48).

Strategy: pure data-parallel over batch across 8 NeuronCores (32 batches per
core, no collectives). All layouts are arranged on the host so the device
kernel never transposes anything:

  - x is sent transposed per batch: xT [nb, C, T].
  - QK projection computes q^T / k^T in "feature-major" layout [o', tokens]
    with heads padded to 64-row blocks, augmented with 2 extra contraction
    rows that carry the q/k bias cross terms, so scores come out exactly
    (up to a per-head constant, which softmax ignores).  K_contract = 50.
  - Scores are computed transposed, S^T[k, q], per head with 2-head row-tiled
    matmul concurrency (head dim 50 <= 64 rows).
  - Softmax skips the max-subtraction (inputs are well-scaled gaussians),
    exp on ACT straight PSUM->SBUF, causal mask applied multiplicatively
    afterwards with 0/1 triangular tiles (DVE + GpSimd split).
  - The PV matmul consumes V in token-major layout (computed directly by
    swapping stationary/moving operands - no transpose), augmented with a
    leading ones column per head so row 0 of each head block is the softmax
    denominator. Col-tiled 2-head concurrency.
  - Normalization: reciprocal of the denominator rows, GpSimd
    partition-broadcast, one fused multiply that also moves PSUM->SBUF.
  - Output projection consumes the attention output directly in its
    [c', token] layout; V-bias is folded into the projection bias on host.

Matmuls run in bf16 by default (fp32 PSUM accumulation; rel err ~4.5e-3 vs
~1.7e-3 for the float32r variant, but ~3x faster: fp32r cannot amortize its
serialized 4-byte weight loads, and walrus rejects fp32r column tiling).
Set KERNEL_MM_BF16=0 for the float32r variant.
"""

import os
import sys

import numpy as np

try:
    import ml_dtypes

    BF16_NP = ml_dtypes.bfloat16
except ImportError:  # pragma: no cover
    BF16_NP = None

for _p in ("/opt/trn_rl_repo",):
    if os.path.isdir(_p) and _p not in sys.path:
        sys.path.insert(0, _p)

from contextlib import ExitStack

import concourse.bass as bass
import concourse.bacc as bacc
import concourse.tile as tile
from concourse import mybir
from concourse.bass_utils import run_bass_kernel_spmd

P = 128
T = 256
C = 384
NH = 8
HD = 48
KA = 50  # augmented contraction rows per head (48 + cq/ck row + ones row)
HB = 64  # padded head block stride
DT = mybir.dt.float32
DTR = mybir.dt.float32r
BF = mybir.dt.bfloat16
AF = mybir.ActivationFunctionType
N_CORES = 8
B_FULL = 256
NB = B_FULL // N_CORES  # batches per core

EPS_PAD = 1e-20  # value for padded V columns (keeps reciprocal finite)

MM_BF16 = os.environ.get("KERNEL_MM_BF16", "1") == "1"
MMDT = BF if MM_BF16 else DTR
MMNP = None  # resolved lazily (BF16_NP) in make_consts/make_in_maps


def build_nc(nb: int = NB, debug: bool = False, repeat: int = 1, timing: bool = False):
    nc = bacc.Bacc(None)

    xT = nc.declare_dram_parameter("xT", [nb, C, T], MMDT, isOutput=False)
    wqk_d = nc.declare_dram_parameter("wqk", [3, P, 1024], MMDT, isOutput=False)
    wv_d = nc.declare_dram_parameter("wv", [3, P, 512], MMDT, isOutput=False)
    wp_d = nc.declare_dram_parameter("wp", [4, P, 384], MMDT, isOutput=False)
    bq_d = nc.declare_dram_parameter("bq", [P, 1], DT, isOutput=False)
    bk_d = nc.declare_dram_parameter("bk", [P, 1], DT, isOutput=False)
    tri0_d = nc.declare_dram_parameter("tri0", [P, 256], BF, isOutput=False)
    tri1_d = nc.declare_dram_parameter("tri1", [P, 512], BF, isOutput=False)
    bp_d = nc.declare_dram_parameter("bp", [P, 384], DT, isOutput=False)
    vinit_d = nc.declare_dram_parameter("vinit", [P, 1024], BF, isOutput=False)
    if timing:
        y_d = nc.dram_tensor("y_int", [nb, T, C], DT)
        ydum_d = nc.declare_dram_parameter("ydum", [P, 4], DT, isOutput=True)
    else:
        y_d = nc.declare_dram_parameter("y", [nb, T, C], DT, isOutput=True)

    with tile.TileContext(nc) as tc, ExitStack() as ctx:
        const = ctx.enter_context(tc.tile_pool(name="const", bufs=1))
        xtp = ctx.enter_context(tc.tile_pool(name="xt", bufs=6))
        qkp = ctx.enter_context(tc.tile_pool(name="qkt", bufs=2))
        vp = ctx.enter_context(tc.tile_pool(name="v", bufs=2))
        ptp = ctx.enter_context(tc.tile_pool(name="pt", bufs=2))
        aop = ctx.enter_context(tc.tile_pool(name="ao", bufs=2))
        yp = ctx.enter_context(tc.tile_pool(name="y", bufs=4))
        psA = ctx.enter_context(
            tc.tile_pool(name="psA", bufs=1, space=bass.MemorySpace.PSUM)
        )
        psV = ctx.enter_context(
            tc.tile_pool(name="psV", bufs=1, space=bass.MemorySpace.PSUM)
        )
        psS = ctx.enter_context(
            tc.tile_pool(name="psS", bufs=1, space=bass.MemorySpace.PSUM)
        )
        psO = ctx.enter_context(
            tc.tile_pool(name="psO", bufs=1, space=bass.MemorySpace.PSUM)
        )
        psY = ctx.enter_context(
            tc.tile_pool(name="psY", bufs=1, space=bass.MemorySpace.PSUM)
        )

        # ---- load constants ------------------------------------------------
        wqk_sb = []
        wv_sb = []
        wp_sb = []
        for ci in range(3):
            t = const.tile([P, 1024], MMDT, tag=f"wqk{ci}")
            nc.sync.dma_start(t[:], wqk_d[ci])
            wqk_sb.append(t)
        for ci in range(3):
            t = const.tile([P, 512], MMDT, tag=f"wv{ci}")
            nc.sync.dma_start(t[:], wv_d[ci])
            wv_sb.append(t)
        for cc in range(4):
            t = const.tile([P, 384], MMDT, tag=f"wp{cc}")
            nc.sync.dma_start(t[:], wp_d[cc])
            wp_sb.append(t)
        bq_sb = const.tile([P, 1], DT, tag="bq")
        nc.sync.dma_start(bq_sb[:], bq_d[:])
        bk_sb = const.tile([P, 1], DT, tag="bk")
        nc.sync.dma_start(bk_sb[:], bk_d[:])
        tri0_sb = const.tile([P, 256], BF, tag="tri0")
        nc.sync.dma_start(tri0_sb[:], tri0_d[:])
        tri1_sb = const.tile([P, 512], BF, tag="tri1")
        nc.sync.dma_start(tri1_sb[:], tri1_d[:])
        bp_sb = const.tile([P, 384], DT, tag="bp")
        nc.sync.dma_start(bp_sb[:], bp_d[:])

        tri0_r = tri0_sb[:].rearrange("p (j r) -> p j r", r=128)
        tri1_r = tri1_sb[:].rearrange("p (j r) -> p j r", r=256)

        # ---- per-batch-pair pipeline ---------------------------------------
        assert nb % 2 == 0
        for bp_it in range((nb // 2) * repeat):
            b0 = 2 * (bp_it % (nb // 2))
            xt = []
            for ci in range(3):
                t = xtp.tile([P, 2 * T], MMDT, tag="xt")
                nc.sync.dma_start(
                    t[:].rearrange("p (b t) -> p b t", b=2),
                    xT[b0 : b0 + 2, 128 * ci : 128 * ci + 128, :].rearrange(
                        "b p t -> p b t"
                    ),
                )
                xt.append(t)

            # QK projection: 2 waves x 2 half-waves, each [128, 1024] psum
            qk_sb = {}
            for w, name in ((0, "qt"), (1, "kt")):
                dst = qkp.tile([P, 2048], MMDT, tag=name)
                for half in range(2):
                    ps = psA.tile([P, 1024], DT, tag="psA")
                    for oc2 in range(2):
                        oc = 2 * half + oc2
                        for ci in range(3):
                            nc.tensor.matmul(
                                ps[:, 512 * oc2 : 512 * oc2 + 512],
                                wqk_sb[ci][
                                    :, 512 * w + 128 * oc : 512 * w + 128 * oc + 128
                                ],
                                xt[ci][:],
                                start=(ci == 0),
                                stop=(ci == 2),
                            )
                    bias = bq_sb if w == 0 else bk_sb
                    if (half + w) % 2 == 0:
                        nc.vector.tensor_scalar_add(
                            dst[:, 1024 * half : 1024 * half + 1024],
                            ps[:],
                            bias[:, 0:1],
                        )
                    else:
                        nc.scalar.activation(
                            dst[:, 1024 * half : 1024 * half + 1024],
                            ps[:],
                            AF.Identity,
                            bias=bias[:, 0:1],
                        )
                qk_sb[name] = dst
            qt, kt = qk_sb["qt"], qk_sb["kt"]

            # V in token-major layout (per batch), ones column + eps pads
            v_sbs = []
            for bb in range(2):
                vt = vp.tile([P, 1024], BF, tag="v")
                nc.sync.dma_start(vt[:], vinit_d[:])
                v_sbs.append(vt)
            for tch in range(4):
                bb, tcx = tch // 2, tch % 2
                psv = psV.tile([P, 512], DT, tag="psV")
                for ci in range(3):
                    nc.tensor.matmul(
                        psv[:],
                        xt[ci][:, 256 * bb + 128 * tcx : 256 * bb + 128 * tcx + 128],
                        wv_sb[ci][:],
                        start=(ci == 0),
                        stop=(ci == 2),
                    )
                half = v_sbs[bb][:, 512 * tcx : 512 * tcx + 512].rearrange(
                    "p (h c) -> p h c", c=HB
                )
                psv_r = psv[:].rearrange("p (h c) -> p h c", c=HB)
                nc.scalar.activation(half[:, :, 1:49], psv_r[:, :, 0:48], AF.Copy)

            for bb in range(2):
                b = b0 + bb
                v_sb = v_sbs[bb]
                # S^T per head pair + exp + mask + PV, interleaved per group
                pt = ptp.tile([P, 4096], BF, tag="pt")
                pso = psO.tile([P, 1024], DT, tag="psO")
                for g in range(4):
                    pss = psS.tile([P, 1024], DT, tag="psS")
                    for kx in range(2):
                        for j in range(2):
                            base = HB * j
                            nc.tensor.matmul(
                                pss[:, 512 * j + 256 * kx : 512 * j + 256 * kx + 256],
                                kt[
                                    base : base + KA,
                                    512 * g + 256 * bb + 128 * kx : 512 * g
                                    + 256 * bb
                                    + 128 * kx
                                    + 128,
                                ],
                                qt[
                                    base : base + KA,
                                    512 * g + 256 * bb : 512 * g + 256 * bb + 256,
                                ],
                                start=True,
                                stop=True,
                            )
                    nc.scalar.activation(
                        pt[:, 1024 * g : 1024 * g + 1024], pss[:], AF.Exp
                    )
                    pg = pt[:, 1024 * g : 1024 * g + 1024].rearrange(
                        "p (j r) -> p j r", r=512
                    )
                    nc.vector.tensor_mul(pg[:, :, 0:128], pg[:, :, 0:128], tri0_r)
                    nc.gpsimd.tensor_mul(
                        pg[:, :, 256:512], pg[:, :, 256:512], tri1_r
                    )
                    # PV for this group
                    for j in range(2):
                        for kx in range(2):
                            h = 2 * g + j
                            nc.tensor.matmul(
                                pso[HB * j : HB * j + HB, 256 * g : 256 * g + 256],
                                v_sb[:, 512 * kx + HB * h : 512 * kx + HB * h + HB],
                                pt[
                                    :,
                                    1024 * g
                                    + 512 * j
                                    + 256 * kx : 1024 * g
                                    + 512 * j
                                    + 256 * kx
                                    + 256,
                                ],
                                start=(kx == 0),
                                stop=(kx == 1),
                                tile_position=(0, HB * j),
                            )

                rec_e = aop.tile([1, 1024], DT, tag="rece")
                rec_o = aop.tile([1, 1024], DT, tag="reco")
                nc.vector.reciprocal(rec_e[0:1, :], pso[0:1, :])
                nc.vector.reciprocal(rec_o[0:1, :], pso[64:65, :])
                # partition_broadcast ucode: source must be cpu0-readable
                # (partition 0) and dest must start at partition 0.
                denb = aop.tile([P, 1024], DT, tag="denb")
                nc.gpsimd.partition_broadcast(denb[:, :], rec_o[0:1, :])
                nc.gpsimd.partition_broadcast(denb[0:64, :], rec_e[0:1, :])
                ao = aop.tile([P, 1024], MMDT, tag="ao")
                nc.vector.tensor_mul(ao[:], pso[:], denb[:])

                # output projection
                for tcx in range(2):
                    psy = psY.tile([P, 384], DT, tag="psY")
                    for cc in range(4):
                        nc.tensor.matmul(
                            psy[:],
                            ao[:, 256 * cc + 128 * tcx : 256 * cc + 128 * tcx + 128],
                            wp_sb[cc][:],
                            start=(cc == 0),
                            stop=(cc == 3),
                        )
                    ysb = yp.tile([P, 384], DT, tag="y")
                    nc.vector.tensor_add(ysb[:], psy[:], bp_sb[:])
                    nc.sync.dma_start(
                        y_d[b, 128 * tcx : 128 * tcx + 128, :], ysb[:]
                    )

        if timing:
            nc.sync.dma_start(ydum_d[:], bp_sb[:, 0:4])

    nc.compile()
    return nc


def make_consts(attn_w, attn_b, proj_w, proj_b):
    attn_w = np.asarray(attn_w, dtype=np.float32)
    attn_b = np.asarray(attn_b, dtype=np.float32)
    proj_w = np.asarray(proj_w, dtype=np.float32)
    proj_b = np.asarray(proj_b, dtype=np.float32)

    s = 1.0 / np.sqrt(HD)
    Wq, Wk, Wv = attn_w[0:C], attn_w[C : 2 * C], attn_w[2 * C : 3 * C]
    bq, bk, bv = attn_b[0:C], attn_b[C : 2 * C], attn_b[2 * C : 3 * C]

    # WQK: [C, 1024] -> [3, 128, 1024]
    M = np.zeros((C, 1024), dtype=np.float32)
    for h in range(NH):
        Wq_h = Wq[HD * h : HD * h + HD]  # [48, C]
        Wk_h = Wk[HD * h : HD * h + HD]
        bq_h = bq[HD * h : HD * h + HD]
        bk_h = bk[HD * h : HD * h + HD]
        # q-hat block
        M[:, HB * h : HB * h + HD] = (s * Wq_h).T
        M[:, HB * h + 48] = s * (bk_h @ Wq_h)  # c_q row
        # (row 49 of q-hat is the ones row via bias)
        # k-hat block
        M[:, 512 + HB * h : 512 + HB * h + HD] = Wk_h.T
        # (row 48 of k-hat is the ones row via bias)
        M[:, 512 + HB * h + 49] = s * (bq_h @ Wk_h)  # c_k row
    WQK = np.ascontiguousarray(M.reshape(C, 1024).reshape(3, P, 1024))

    # WV: [C, 512] -> [3, 128, 512]; col HB*h+j (j<48) = Wv row HD*h+j
    V = np.zeros((C, 512), dtype=np.float32)
    for h in range(NH):
        V[:, HB * h : HB * h + HD] = Wv[HD * h : HD * h + HD].T
    WV = np.ascontiguousarray(V.reshape(3, P, 512))

    # WP: [512, 384] -> [4, 128, 384]; row HB*h + 1 + j = proj_w[:, HD*h+j]
    Wp_aug = np.zeros((512, C), dtype=np.float32)
    for h in range(NH):
        Wp_aug[HB * h + 1 : HB * h + 1 + HD, :] = proj_w[:, HD * h : HD * h + HD].T
    WP = np.ascontiguousarray(Wp_aug.reshape(4, P, 384))

    BQ = np.zeros((P, 1), dtype=np.float32)
    BQ[49, 0] = 1.0
    BQ[49 + HB, 0] = 1.0
    BK = np.zeros((P, 1), dtype=np.float32)
    BK[48, 0] = 1.0
    BK[48 + HB, 0] = 1.0

    # causal 0/1 masks for S^T[k, q] tiles (repeated x2 for head pairs)
    kk = np.arange(128)[:, None]
    qq = np.arange(128)[None, :]
    tri = (qq >= kk).astype(np.float32)  # [128k, 128q]
    TRI0 = np.ascontiguousarray(np.concatenate([tri, tri], axis=1)).astype(BF16_NP)
    t1 = np.concatenate([np.zeros((128, 128), np.float32), tri], axis=1)  # [128,256]
    TRI1 = np.ascontiguousarray(np.concatenate([t1, t1], axis=1)).astype(BF16_NP)

    bp_eff = proj_b + proj_w @ bv
    BP = np.ascontiguousarray(np.broadcast_to(bp_eff[None, :], (P, 384))).astype(
        np.float32
    )

    # v-init pattern: ones column at 64h, EPS_PAD at cols 49..63 of each block
    vinit_row = np.zeros(1024, dtype=np.float32)
    for kx in range(2):
        for h in range(NH):
            off = 512 * kx + HB * h
            vinit_row[off] = 1.0
            vinit_row[off + 49 : off + HB] = EPS_PAD
    VINIT = np.ascontiguousarray(np.broadcast_to(vinit_row[None, :], (P, 1024))).astype(
        BF16_NP
    )

    mmnp = BF16_NP if MM_BF16 else np.float32
    WQK = WQK.astype(mmnp)
    WV = WV.astype(mmnp)
    WP = WP.astype(mmnp)

    return {
        "vinit": VINIT,
        "wqk": WQK,
        "wv": WV,
        "wp": WP,
        "bq": BQ,
        "bk": BK,
        "tri0": TRI0,
        "tri1": TRI1,
        "bp": BP,
    }


_NC_CACHE = {}


def get_nc(nb: int = NB):
    if nb not in _NC_CACHE:
        _NC_CACHE[nb] = build_nc(nb)
    return _NC_CACHE[nb]


def make_in_maps(x, attn_w, attn_b, proj_w, proj_b):
    x = np.asarray(x, dtype=np.float32)
    consts = make_consts(attn_w, attn_b, proj_w, proj_b)
    in_maps = []
    for core in range(N_CORES):
        xs = x[core * NB : (core + 1) * NB]  # [NB, T, C]
        xTl = np.ascontiguousarray(xs.transpose(0, 2, 1))  # [NB, C, T]
        if MM_BF16:
            xTl = xTl.astype(BF16_NP)
        m = {"xT": xTl}
        m.update(consts)
        in_maps.append(m)
    return in_maps


def kernel(x, attn_w, attn_b, proj_w, proj_b):
    nc = get_nc(NB)
    in_maps = make_in_maps(x, attn_w, attn_b, proj_w, proj_b)
    res = run_bass_kernel_spmd(nc, in_maps, core_ids=list(range(N_CORES)))
    out = np.concatenate(
        [res.results[i]["y"] for i in range(N_CORES)], axis=0
    ).astype(np.float32)
    return out



# revision 5
# speedup vs baseline: 1.5414x; 1.5414x over previous
"""Causal self-attention Trainium2 kernel (B=256, T=256, C=384, 8 heads x 48).

Strategy: pure data-parallel over batch across 8 NeuronCores (32 batches per
core, no collectives). All layouts are arranged on the host so the device
kernel never transposes anything:

  - x is sent transposed per batch: xT [nb, C, T].
  - QK projection computes q^T / k^T in "feature-major" layout [o', tokens]
    with heads padded to 64-row blocks, augmented with 2 extra contraction
    rows that carry the q/k bias cross terms, so scores come out exactly
    (up to a per-head constant, which softmax ignores).  K_contract = 50.
  - Scores are computed transposed, S^T[k, q], per head with 2-head row-tiled
    matmul concurrency (head dim 50 <= 64 rows).  Causal block-sparsity: the
    (k in 128..255, q in 0..127) quadrant is fully masked, so its matmul,
    exp, mask and PV contributions are skipped entirely.  Per j-half layout:
    [kx0 q0..255 | kx1 q128..255 | dead].
  - Softmax skips the max-subtraction (inputs are well-scaled gaussians),
    exp on ACT straight PSUM->SBUF (strided, skipping the dead region),
    causal mask applied multiplicatively to the two triangular 128-blocks
    (both use the same tri pattern) on DVE.
  - The PV matmul consumes V in token-major layout (computed directly by
    swapping stationary/moving operands - no transpose), augmented with a
    leading ones column per head so row 0 of each head block is the softmax
    denominator.  Col-tiled 2-head concurrency; output per half-batch into a
    single-bank [128, 512] PSUM tile (bufs=2) for pipelining.
  - Normalization per half-batch: strided 2-row reciprocal_approx_fast of
    the denominator rows (~5x faster than vector.reciprocal), GpSimd
    partition-broadcast, one fused multiply PSUM->SBUF.
  - Output projection consumes the attention output directly in its
    [c', token] layout; V-bias is folded into the projection bias on host.

Matmuls run in bf16 (fp32 PSUM accumulation).
"""

import os
import sys

import numpy as np

try:
    import ml_dtypes

    BF16_NP = ml_dtypes.bfloat16
except ImportError:  # pragma: no cover
    BF16_NP = None

for _p in ("/opt/trn_rl_repo",):
    if os.path.isdir(_p) and _p not in sys.path:
        sys.path.insert(0, _p)

from contextlib import ExitStack

import concourse.bass as bass
import concourse.bacc as bacc
import concourse.tile as tile
from concourse import mybir
from concourse.bass_utils import run_bass_kernel_spmd

P = 128
T = 256
C = 384
NH = 8
HD = 48
KA = 50  # augmented contraction rows per head (48 + cq/ck row + ones row)
HB = 64  # padded head block stride
DT = mybir.dt.float32
DTR = mybir.dt.float32r
BF = mybir.dt.bfloat16
AF = mybir.ActivationFunctionType
N_CORES = 8
B_FULL = 256
NB = B_FULL // N_CORES  # batches per core

EPS_PAD = 1e-20  # value for padded V columns (keeps reciprocal finite)

MM_BF16 = os.environ.get("KERNEL_MM_BF16", "1") == "1"
MMDT = BF if MM_BF16 else DTR
FAST_RECIP = os.environ.get("KERNEL_FAST_RECIP", "1") == "1"


def build_nc(nb: int = NB, debug: bool = False, repeat: int = 1, timing: bool = False):
    nc = bacc.Bacc(None)

    xT = nc.declare_dram_parameter("xT", [nb, C, T], MMDT, isOutput=False)
    wqk_d = nc.declare_dram_parameter("wqk", [3, P, 1024], MMDT, isOutput=False)
    wv_d = nc.declare_dram_parameter("wv", [3, P, 512], MMDT, isOutput=False)
    wp_d = nc.declare_dram_parameter("wp", [4, P, 384], MMDT, isOutput=False)
    bq_d = nc.declare_dram_parameter("bq", [P, 1], DT, isOutput=False)
    bk_d = nc.declare_dram_parameter("bk", [P, 1], DT, isOutput=False)
    tri0_d = nc.declare_dram_parameter("tri0", [P, 256], BF, isOutput=False)
    bp_d = nc.declare_dram_parameter("bp", [P, 384], DT, isOutput=False)
    vinit_d = nc.declare_dram_parameter("vinit", [P, 1024], BF, isOutput=False)
    if timing:
        y_d = nc.dram_tensor("y_int", [nb, T, C], DT)
        ydum_d = nc.declare_dram_parameter("ydum", [P, 4], DT, isOutput=True)
    else:
        y_d = nc.declare_dram_parameter("y", [nb, T, C], DT, isOutput=True)

    with tile.TileContext(nc) as tc, ExitStack() as ctx:
        const = ctx.enter_context(tc.tile_pool(name="const", bufs=1))
        xtp = ctx.enter_context(tc.tile_pool(name="xt", bufs=6))
        qkp = ctx.enter_context(tc.tile_pool(name="qkt", bufs=2))
        vp = ctx.enter_context(tc.tile_pool(name="v", bufs=2))
        ptp = ctx.enter_context(tc.tile_pool(name="pt", bufs=2))
        aop = ctx.enter_context(tc.tile_pool(name="ao", bufs=2))
        yp = ctx.enter_context(tc.tile_pool(name="y", bufs=4))
        psA = ctx.enter_context(
            tc.tile_pool(name="psA", bufs=1, space=bass.MemorySpace.PSUM)
        )
        psV = ctx.enter_context(
            tc.tile_pool(name="psV", bufs=1, space=bass.MemorySpace.PSUM)
        )
        psS = ctx.enter_context(
            tc.tile_pool(name="psS", bufs=1, space=bass.MemorySpace.PSUM)
        )
        psO = ctx.enter_context(
            tc.tile_pool(name="psO", bufs=2, space=bass.MemorySpace.PSUM)
        )
        psY = ctx.enter_context(
            tc.tile_pool(name="psY", bufs=1, space=bass.MemorySpace.PSUM)
        )

        # ---- load constants ------------------------------------------------
        wqk_sb = []
        wv_sb = []
        wp_sb = []
        for ci in range(3):
            t = const.tile([P, 1024], MMDT, tag=f"wqk{ci}")
            nc.sync.dma_start(t[:], wqk_d[ci])
            wqk_sb.append(t)
        for ci in range(3):
            t = const.tile([P, 512], MMDT, tag=f"wv{ci}")
            nc.sync.dma_start(t[:], wv_d[ci])
            wv_sb.append(t)
        for cc in range(4):
            t = const.tile([P, 384], MMDT, tag=f"wp{cc}")
            nc.sync.dma_start(t[:], wp_d[cc])
            wp_sb.append(t)
        bq_sb = const.tile([P, 1], DT, tag="bq")
        nc.sync.dma_start(bq_sb[:], bq_d[:])
        bk_sb = const.tile([P, 1], DT, tag="bk")
        nc.sync.dma_start(bk_sb[:], bk_d[:])
        tri0_sb = const.tile([P, 256], BF, tag="tri0")
        nc.sync.dma_start(tri0_sb[:], tri0_d[:])
        bp_sb = const.tile([P, 384], DT, tag="bp")
        nc.sync.dma_start(bp_sb[:], bp_d[:])

        tri0_r = tri0_sb[:].rearrange("p (j r) -> p j r", r=128)

        # ---- per-batch-pair pipeline ---------------------------------------
        assert nb % 2 == 0
        for bp_it in range((nb // 2) * repeat):
            b0 = 2 * (bp_it % (nb // 2))
            xt = []
            for ci in range(3):
                t = xtp.tile([P, 2 * T], MMDT, tag="xt")
                nc.sync.dma_start(
                    t[:].rearrange("p (b t) -> p b t", b=2),
                    xT[b0 : b0 + 2, 128 * ci : 128 * ci + 128, :].rearrange(
                        "b p t -> p b t"
                    ),
                )
                xt.append(t)

            # QK projection: 2 waves x 2 half-waves, each [128, 1024] psum
            qk_sb = {}
            for w, name in ((0, "qt"), (1, "kt")):
                dst = qkp.tile([P, 2048], MMDT, tag=name)
                for half in range(2):
                    ps = psA.tile([P, 1024], DT, tag="psA")
                    for oc2 in range(2):
                        oc = 2 * half + oc2
                        for ci in range(3):
                            nc.tensor.matmul(
                                ps[:, 512 * oc2 : 512 * oc2 + 512],
                                wqk_sb[ci][
                                    :, 512 * w + 128 * oc : 512 * w + 128 * oc + 128
                                ],
                                xt[ci][:],
                                start=(ci == 0),
                                stop=(ci == 2),
                            )
                    bias = bq_sb if w == 0 else bk_sb
                    if (half + w) % 2 == 0:
                        nc.vector.tensor_scalar_add(
                            dst[:, 1024 * half : 1024 * half + 1024],
                            ps[:],
                            bias[:, 0:1],
                        )
                    else:
                        nc.scalar.activation(
                            dst[:, 1024 * half : 1024 * half + 1024],
                            ps[:],
                            AF.Identity,
                            bias=bias[:, 0:1],
                        )
                qk_sb[name] = dst
            qt, kt = qk_sb["qt"], qk_sb["kt"]

            # V in token-major layout (per batch), ones column + eps pads
            v_sbs = []
            for bb in range(2):
                vt = vp.tile([P, 1024], BF, tag="v")
                nc.sync.dma_start(vt[:], vinit_d[:])
                v_sbs.append(vt)
            for tch in range(4):
                bb, tcx = tch // 2, tch % 2
                psv = psV.tile([P, 512], DT, tag="psV")
                for ci in range(3):
                    nc.tensor.matmul(
                        psv[:],
                        xt[ci][:, 256 * bb + 128 * tcx : 256 * bb + 128 * tcx + 128],
                        wv_sb[ci][:],
                        start=(ci == 0),
                        stop=(ci == 2),
                    )
                half = v_sbs[bb][:, 512 * tcx : 512 * tcx + 512].rearrange(
                    "p (h c) -> p h c", c=HB
                )
                psv_r = psv[:].rearrange("p (h c) -> p h c", c=HB)
                nc.scalar.activation(half[:, :, 1:49], psv_r[:, :, 0:48], AF.Copy)

            for bb in range(2):
                b = b0 + bb
                v_sb = v_sbs[bb]
                # S^T per head pair + exp + mask + PV, per half-batch psO
                pt = ptp.tile([P, 4096], BF, tag="pt")
                ao = aop.tile([P, 1024], MMDT, tag="ao")
                for h2 in range(2):
                    psoh = psO.tile([P, 512], DT, tag="psO")
                    for gg in range(2):
                        g = 2 * h2 + gg
                        pss = psS.tile([P, 1024], DT, tag="psS")
                        qb = 512 * g + 256 * bb
                        for j in range(2):
                            base = HB * j
                            # kx0: keys 0..127, all 256 queries
                            nc.tensor.matmul(
                                pss[:, 512 * j : 512 * j + 256],
                                kt[base : base + KA, qb : qb + 128],
                                qt[base : base + KA, qb : qb + 256],
                                start=True,
                                stop=True,
                            )
                            # kx1: keys 128..255, queries 128..255 only
                            # (q<128 is fully causal-masked for these keys)
                            nc.tensor.matmul(
                                pss[:, 512 * j + 256 : 512 * j + 384],
                                kt[base : base + KA, qb + 128 : qb + 256],
                                qt[base : base + KA, qb + 128 : qb + 256],
                                start=True,
                                stop=True,
                            )
                        pt_g = pt[:, 1024 * g : 1024 * g + 1024].rearrange(
                            "p (j r) -> p j r", r=512
                        )
                        pss_r = pss[:].rearrange("p (j r) -> p j r", r=512)
                        nc.scalar.activation(
                            pt_g[:, :, 0:384], pss_r[:, :, 0:384], AF.Exp
                        )
                        # triangular masks: diagonal blocks of kx0 (cols
                        # 0..127) and kx1 (cols 256..383) share the pattern
                        nc.vector.tensor_mul(
                            pt_g[:, :, 0:128], pt_g[:, :, 0:128], tri0_r
                        )
                        nc.vector.tensor_mul(
                            pt_g[:, :, 256:384], pt_g[:, :, 256:384], tri0_r
                        )
                        # PV for this group
                        for j in range(2):
                            h = 2 * g + j
                            pc = 1024 * g + 512 * j
                            nc.tensor.matmul(
                                psoh[HB * j : HB * j + HB, 256 * gg : 256 * gg + 256],
                                v_sb[:, HB * h : HB * h + HB],
                                pt[:, pc : pc + 256],
                                start=True,
                                stop=False,
                                tile_position=(0, HB * j),
                            )
                            nc.tensor.matmul(
                                psoh[
                                    HB * j : HB * j + HB,
                                    256 * gg + 128 : 256 * gg + 256,
                                ],
                                v_sb[:, 512 + HB * h : 512 + HB * h + HB],
                                pt[:, pc + 256 : pc + 384],
                                start=False,
                                stop=True,
                                tile_position=(0, HB * j),
                            )

                    # normalize this half-batch: rows 0 / 64 hold denominators
                    rec = aop.tile([1, 1024], DT, tag="rec")
                    recip_op = (
                        nc.vector.reciprocal_approx_fast
                        if FAST_RECIP
                        else nc.vector.reciprocal
                    )
                    recip_op(rec[0:1, 0:512], psoh[0:1, :])
                    recip_op(rec[0:1, 512:1024], psoh[64:65, :])
                    denb = aop.tile([P, 512], DT, tag="denb")
                    nc.gpsimd.partition_broadcast(denb[:, :], rec[0:1, 512:1024])
                    nc.gpsimd.partition_broadcast(denb[0:64, :], rec[0:1, 0:512])
                    nc.vector.tensor_mul(
                        ao[:, 512 * h2 : 512 * h2 + 512], psoh[:], denb[:]
                    )

                # output projection
                for tcx in range(2):
                    psy = psY.tile([P, 384], DT, tag="psY")
                    for cc in range(4):
                        nc.tensor.matmul(
                            psy[:],
                            ao[:, 256 * cc + 128 * tcx : 256 * cc + 128 * tcx + 128],
                            wp_sb[cc][:],
                            start=(cc == 0),
                            stop=(cc == 3),
                        )
                    ysb = yp.tile([P, 384], DT, tag="y")
                    nc.vector.tensor_add(ysb[:], psy[:], bp_sb[:])
                    nc.sync.dma_start(
                        y_d[b, 128 * tcx : 128 * tcx + 128, :], ysb[:]
                    )

        if timing:
            nc.sync.dma_start(ydum_d[:], bp_sb[:, 0:4])

    nc.compile()
    return nc


def make_consts(attn_w, attn_b, proj_w, proj_b):
    attn_w = np.asarray(attn_w, dtype=np.float32)
    attn_b = np.asarray(attn_b, dtype=np.float32)
    proj_w = np.asarray(proj_w, dtype=np.float32)
    proj_b = np.asarray(proj_b, dtype=np.float32)

    s = 1.0 / np.sqrt(HD)
    Wq, Wk, Wv = attn_w[0:C], attn_w[C : 2 * C], attn_w[2 * C : 3 * C]
    bq, bk, bv = attn_b[0:C], attn_b[C : 2 * C], attn_b[2 * C : 3 * C]

    # WQK: [C, 1024] -> [3, 128, 1024]
    M = np.zeros((C, 1024), dtype=np.float32)
    for h in range(NH):
        Wq_h = Wq[HD * h : HD * h + HD]  # [48, C]
        Wk_h = Wk[HD * h : HD * h + HD]
        bq_h = bq[HD * h : HD * h + HD]
        bk_h = bk[HD * h : HD * h + HD]
        # q-hat block
        M[:, HB * h : HB * h + HD] = (s * Wq_h).T
        M[:, HB * h + 48] = s * (bk_h @ Wq_h)  # c_q row
        # (row 49 of q-hat is the ones row via bias)
        # k-hat block
        M[:, 512 + HB * h : 512 + HB * h + HD] = Wk_h.T
        # (row 48 of k-hat is the ones row via bias)
        M[:, 512 + HB * h + 49] = s * (bq_h @ Wk_h)  # c_k row
    WQK = np.ascontiguousarray(M.reshape(C, 1024).reshape(3, P, 1024))

    # WV: [C, 512] -> [3, 128, 512]; col HB*h+j (j<48) = Wv row HD*h+j
    V = np.zeros((C, 512), dtype=np.float32)
    for h in range(NH):
        V[:, HB * h : HB * h + HD] = Wv[HD * h : HD * h + HD].T
    WV = np.ascontiguousarray(V.reshape(3, P, 512))

    # WP: [512, 384] -> [4, 128, 384]; row HB*h + 1 + j = proj_w[:, HD*h+j]
    Wp_aug = np.zeros((512, C), dtype=np.float32)
    for h in range(NH):
        Wp_aug[HB * h + 1 : HB * h + 1 + HD, :] = proj_w[:, HD * h : HD * h + HD].T
    WP = np.ascontiguousarray(Wp_aug.reshape(4, P, 384))

    BQ = np.zeros((P, 1), dtype=np.float32)
    BQ[49, 0] = 1.0
    BQ[49 + HB, 0] = 1.0
    BK = np.zeros((P, 1), dtype=np.float32)
    BK[48, 0] = 1.0
    BK[48 + HB, 0] = 1.0

    # causal 0/1 mask for S^T[k, q] diagonal tiles (repeated x2 for head pairs)
    kk = np.arange(128)[:, None]
    qq = np.arange(128)[None, :]
    tri = (qq >= kk).astype(np.float32)  # [128k, 128q]
    TRI0 = np.ascontiguousarray(np.concatenate([tri, tri], axis=1)).astype(BF16_NP)

    bp_eff = proj_b + proj_w @ bv
    BP = np.ascontiguousarray(np.broadcast_to(bp_eff[None, :], (P, 384))).astype(
        np.float32
    )

    # v-init pattern: ones column at 64h, EPS_PAD at cols 49..63 of each block
    vinit_row = np.zeros(1024, dtype=np.float32)
    for kx in range(2):
        for h in range(NH):
            off = 512 * kx + HB * h
            vinit_row[off] = 1.0
            vinit_row[off + 49 : off + HB] = EPS_PAD
    VINIT = np.ascontiguousarray(np.broadcast_to(vinit_row[None, :], (P, 1024))).astype(
        BF16_NP
    )

    mmnp = BF16_NP if MM_BF16 else np.float32
    WQK = WQK.astype(mmnp)
    WV = WV.astype(mmnp)
    WP = WP.astype(mmnp)

    return {
        "vinit": VINIT,
        "wqk": WQK,
        "wv": WV,
        "wp": WP,
        "bq": BQ,
        "bk": BK,
        "tri0": TRI0,
        "bp": BP,
    }


_NC_CACHE = {}


def get_nc(nb: int = NB):
    if nb not in _NC_CACHE:
        _NC_CACHE[nb] = build_nc(nb)
    return _NC_CACHE[nb]


def make_in_maps(x, attn_w, attn_b, proj_w, proj_b):
    x = np.asarray(x, dtype=np.float32)
    consts = make_consts(attn_w, attn_b, proj_w, proj_b)
    in_maps = []
    for core in range(N_CORES):
        xs = x[core * NB : (core + 1) * NB]  # [NB, T, C]
        xTl = np.ascontiguousarray(xs.transpose(0, 2, 1))  # [NB, C, T]
        if MM_BF16:
            xTl = xTl.astype(BF16_NP)
        m = {"xT": xTl}
        m.update(consts)
        in_maps.append(m)
    return in_maps


def kernel(x, attn_w, attn_b, proj_w, proj_b):
    nc = get_nc(NB)
    in_maps = make_in_maps(x, attn_w, attn_b, proj_w, proj_b)
    res = run_bass_kernel_spmd(nc, in_maps, core_ids=list(range(N_CORES)))
    out = np.concatenate(
        [res.results[i]["y"] for i in range(N_CORES)], axis=0
    ).astype(np.float32)
    return out


# revision 7
# speedup vs baseline: 2.7979x; 1.8152x over previous
"""Causal self-attention Trainium2 kernel (B=256, T=256, C=384, 8 heads x 48).

Strategy: pure data-parallel over batch across 8 NeuronCores (32 batches per
core, no collectives). All layouts are arranged on the host so the device
kernel never transposes anything:

  - x is sent transposed per batch: xT [nb, C, T].
  - QK projection computes q^T / k^T in "feature-major" layout [o', tokens]
    with heads padded to 64-row blocks, augmented with 2 extra contraction
    rows that carry the q/k bias cross terms, so scores come out exactly
    (up to a per-head constant, which softmax ignores).  K_contract = 50.
  - Scores are computed transposed, S^T[k, q], per head with 2-head row-tiled
    matmul concurrency (head dim 50 <= 64 rows).  Causal block-sparsity: the
    (k in 128..255, q in 0..127) quadrant is fully masked, so its matmul,
    exp, mask and PV contributions are skipped entirely.  Per j-half layout:
    [kx0 q0..255 | kx1 q128..255 | dead].
  - Softmax skips the max-subtraction (inputs are well-scaled gaussians),
    exp on ACT straight PSUM->SBUF (strided, skipping the dead region),
    causal mask applied multiplicatively to the two triangular 128-blocks
    (both use the same tri pattern) on DVE.
  - The PV matmul consumes V in token-major layout (computed directly by
    swapping stationary/moving operands - no transpose), augmented with a
    leading ones column per head so row 0 of each head block is the softmax
    denominator.  Col-tiled 2-head concurrency; output per half-batch into a
    single-bank [128, 512] PSUM tile (bufs=2) for pipelining.
  - Normalization per half-batch: strided 2-row reciprocal_approx_fast of
    the denominator rows (~5x faster than vector.reciprocal), GpSimd
    partition-broadcast, one fused multiply PSUM->SBUF.
  - Output projection consumes the attention output directly in its
    [c', token] layout; V-bias is folded into the projection bias on host.

Matmuls run in bf16 (fp32 PSUM accumulation).
"""

import os
import sys

import numpy as np

try:
    import ml_dtypes

    BF16_NP = ml_dtypes.bfloat16
except ImportError:  # pragma: no cover
    BF16_NP = None

for _p in ("/opt/trn_rl_repo",):
    if os.path.isdir(_p) and _p not in sys.path:
        sys.path.insert(0, _p)

from contextlib import ExitStack

import concourse.bass as bass
import concourse.bacc as bacc
import concourse.tile as tile
from concourse import mybir
from concourse.bass_utils import run_bass_kernel_spmd

P = 128
T = 256
C = 384
NH = 8
HD = 48
KA = 50  # augmented contraction rows per head (48 + cq/ck row + ones row)
HB = 64  # padded head block stride
DT = mybir.dt.float32
DTR = mybir.dt.float32r
BF = mybir.dt.bfloat16
AF = mybir.ActivationFunctionType
N_CORES = 8
B_FULL = 256
NB = B_FULL // N_CORES  # batches per core

EPS_PAD = 1e-20  # value for padded V columns (keeps reciprocal finite)

MM_BF16 = os.environ.get("KERNEL_MM_BF16", "1") == "1"
MMDT = BF if MM_BF16 else DTR
FAST_RECIP = os.environ.get("KERNEL_FAST_RECIP", "1") == "1"


def build_nc(nb: int = NB, debug: bool = False, repeat: int = 1, timing: bool = False):
    nc = bacc.Bacc(None)

    xT = nc.declare_dram_parameter("xT", [nb, C, T], MMDT, isOutput=False)
    wqk_d = nc.declare_dram_parameter("wqk", [3, P, 1024], MMDT, isOutput=False)
    wv_d = nc.declare_dram_parameter("wv", [3, P, 512], MMDT, isOutput=False)
    wp_d = nc.declare_dram_parameter("wp", [4, P, 384], MMDT, isOutput=False)
    bq_d = nc.declare_dram_parameter("bq", [P, 1], DT, isOutput=False)
    bk_d = nc.declare_dram_parameter("bk", [P, 1], DT, isOutput=False)
    tri0_d = nc.declare_dram_parameter("tri0", [P, 256], BF, isOutput=False)
    bp_d = nc.declare_dram_parameter("bp", [P, 384], DT, isOutput=False)
    vinit_d = nc.declare_dram_parameter("vinit", [P, 1024], BF, isOutput=False)
    if timing:
        y_d = nc.dram_tensor("y_int", [nb, T, C], DT)
        ydum_d = nc.declare_dram_parameter("ydum", [P, 4], DT, isOutput=True)
    else:
        y_d = nc.declare_dram_parameter("y", [nb, T, C], DT, isOutput=True)

    with tile.TileContext(nc) as tc, ExitStack() as ctx:
        const = ctx.enter_context(tc.tile_pool(name="const", bufs=1))
        xtp = ctx.enter_context(tc.tile_pool(name="xt", bufs=6))
        qkp = ctx.enter_context(tc.tile_pool(name="qkt", bufs=2))
        vp = ctx.enter_context(tc.tile_pool(name="v", bufs=2))
        ptp = ctx.enter_context(tc.tile_pool(name="pt", bufs=2))
        aop = ctx.enter_context(tc.tile_pool(name="ao", bufs=2))
        yp = ctx.enter_context(tc.tile_pool(name="y", bufs=4))
        psA = ctx.enter_context(
            tc.tile_pool(name="psA", bufs=1, space=bass.MemorySpace.PSUM)
        )
        psV = ctx.enter_context(
            tc.tile_pool(name="psV", bufs=1, space=bass.MemorySpace.PSUM)
        )
        psS = ctx.enter_context(
            tc.tile_pool(name="psS", bufs=1, space=bass.MemorySpace.PSUM)
        )
        psO = ctx.enter_context(
            tc.tile_pool(name="psO", bufs=2, space=bass.MemorySpace.PSUM)
        )
        psY = ctx.enter_context(
            tc.tile_pool(name="psY", bufs=1, space=bass.MemorySpace.PSUM)
        )

        # ---- load constants ------------------------------------------------
        wqk_sb = []
        wv_sb = []
        wp_sb = []
        for ci in range(3):
            t = const.tile([P, 1024], MMDT, tag=f"wqk{ci}")
            nc.sync.dma_start(t[:], wqk_d[ci])
            wqk_sb.append(t)
        for ci in range(3):
            t = const.tile([P, 512], MMDT, tag=f"wv{ci}")
            nc.sync.dma_start(t[:], wv_d[ci])
            wv_sb.append(t)
        for cc in range(4):
            t = const.tile([P, 384], MMDT, tag=f"wp{cc}")
            nc.sync.dma_start(t[:], wp_d[cc])
            wp_sb.append(t)
        bq_sb = const.tile([P, 1], DT, tag="bq")
        nc.sync.dma_start(bq_sb[:], bq_d[:])
        bk_sb = const.tile([P, 1], DT, tag="bk")
        nc.sync.dma_start(bk_sb[:], bk_d[:])
        tri0_sb = const.tile([P, 256], BF, tag="tri0")
        nc.sync.dma_start(tri0_sb[:], tri0_d[:])
        bp_sb = const.tile([P, 384], DT, tag="bp")
        nc.sync.dma_start(bp_sb[:], bp_d[:])

        tri0_r = tri0_sb[:].rearrange("p (j r) -> p j r", r=128)

        # ---- per-batch-pair pipeline ---------------------------------------
        assert nb % 2 == 0
        for bp_it in range((nb // 2) * repeat):
            b0 = 2 * (bp_it % (nb // 2))
            xt = []
            for ci in range(3):
                t = xtp.tile([P, 2 * T], MMDT, tag="xt")
                nc.sync.dma_start(
                    t[:].rearrange("p (b t) -> p b t", b=2),
                    xT[b0 : b0 + 2, 128 * ci : 128 * ci + 128, :].rearrange(
                        "b p t -> p b t"
                    ),
                )
                xt.append(t)

            # QK projection: 2 waves x 2 half-waves, each [128, 1024] psum
            qk_sb = {}
            for w, name in ((0, "qt"), (1, "kt")):
                dst = qkp.tile([P, 2048], MMDT, tag=name)
                for half in range(2):
                    ps = psA.tile([P, 1024], DT, tag="psA")
                    for oc2 in range(2):
                        oc = 2 * half + oc2
                        for ci in range(3):
                            nc.tensor.matmul(
                                ps[:, 512 * oc2 : 512 * oc2 + 512],
                                wqk_sb[ci][
                                    :, 512 * w + 128 * oc : 512 * w + 128 * oc + 128
                                ],
                                xt[ci][:],
                                start=(ci == 0),
                                stop=(ci == 2),
                            )
                    bias = bq_sb if w == 0 else bk_sb
                    if (half + w) % 2 == 0:
                        nc.vector.tensor_scalar_add(
                            dst[:, 1024 * half : 1024 * half + 1024],
                            ps[:],
                            bias[:, 0:1],
                        )
                    else:
                        nc.scalar.activation(
                            dst[:, 1024 * half : 1024 * half + 1024],
                            ps[:],
                            AF.Identity,
                            bias=bias[:, 0:1],
                        )
                qk_sb[name] = dst
            qt, kt = qk_sb["qt"], qk_sb["kt"]

            # V in token-major layout (per batch), ones column + eps pads
            v_sbs = []
            for bb in range(2):
                vt = vp.tile([P, 1024], BF, tag="v")
                nc.sync.dma_start(vt[:], vinit_d[:])
                v_sbs.append(vt)
            for tch in range(4):
                bb, tcx = tch // 2, tch % 2
                psv = psV.tile([P, 512], DT, tag="psV")
                for ci in range(3):
                    nc.tensor.matmul(
                        psv[:],
                        xt[ci][:, 256 * bb + 128 * tcx : 256 * bb + 128 * tcx + 128],
                        wv_sb[ci][:],
                        start=(ci == 0),
                        stop=(ci == 2),
                    )
                half = v_sbs[bb][:, 512 * tcx : 512 * tcx + 512].rearrange(
                    "p (h c) -> p h c", c=HB
                )
                psv_r = psv[:].rearrange("p (h c) -> p h c", c=HB)
                nc.scalar.activation(half[:, :, 1:49], psv_r[:, :, 0:48], AF.Copy)

            for bb in range(2):
                b = b0 + bb
                v_sb = v_sbs[bb]
                # S^T per head pair + exp + mask + PV, per half-batch psO
                pt = ptp.tile([P, 4096], BF, tag="pt")
                ao = aop.tile([P, 1024], MMDT, tag="ao")
                for h2 in range(2):
                    psoh = psO.tile([P, 512], DT, tag="psO")
                    for gg in range(2):
                        g = 2 * h2 + gg
                        pss = psS.tile([P, 1024], DT, tag="psS")
                        qb = 512 * g + 256 * bb
                        for j in range(2):
                            base = HB * j
                            # kx0: keys 0..127, all 256 queries
                            nc.tensor.matmul(
                                pss[:, 512 * j : 512 * j + 256],
                                kt[base : base + KA, qb : qb + 128],
                                qt[base : base + KA, qb : qb + 256],
                                start=True,
                                stop=True,
                            )
                            # kx1: keys 128..255, queries 128..255 only
                            # (q<128 is fully causal-masked for these keys)
                            nc.tensor.matmul(
                                pss[:, 512 * j + 256 : 512 * j + 384],
                                kt[base : base + KA, qb + 128 : qb + 256],
                                qt[base : base + KA, qb + 128 : qb + 256],
                                start=True,
                                stop=True,
                            )
                        pt_g = pt[:, 1024 * g : 1024 * g + 1024].rearrange(
                            "p (j r) -> p j r", r=512
                        )
                        pss_r = pss[:].rearrange("p (j r) -> p j r", r=512)
                        nc.scalar.activation(
                            pt_g[:, :, 0:384], pss_r[:, :, 0:384], AF.Exp
                        )
                        # triangular masks: diagonal blocks of kx0 (cols
                        # 0..127) and kx1 (cols 256..383) share the pattern
                        nc.vector.tensor_mul(
                            pt_g[:, :, 0:128], pt_g[:, :, 0:128], tri0_r
                        )
                        nc.vector.tensor_mul(
                            pt_g[:, :, 256:384], pt_g[:, :, 256:384], tri0_r
                        )
                        # PV for this group
                        for j in range(2):
                            h = 2 * g + j
                            pc = 1024 * g + 512 * j
                            nc.tensor.matmul(
                                psoh[HB * j : HB * j + HB, 256 * gg : 256 * gg + 256],
                                v_sb[:, HB * h : HB * h + HB],
                                pt[:, pc : pc + 256],
                                start=True,
                                stop=False,
                                tile_position=(0, HB * j),
                            )
                            nc.tensor.matmul(
                                psoh[
                                    HB * j : HB * j + HB,
                                    256 * gg + 128 : 256 * gg + 256,
                                ],
                                v_sb[:, 512 + HB * h : 512 + HB * h + HB],
                                pt[:, pc + 256 : pc + 384],
                                start=False,
                                stop=True,
                                tile_position=(0, HB * j),
                            )

                    # normalize this half-batch: rows 0 / 64 hold denominators
                    rec = aop.tile([1, 1024], DT, tag="rec")
                    if FAST_RECIP:
                        # custom-DVE op only works with partition-0 base on
                        # HW; stage the j1 den row (partition 64) to p0 via
                        # an ACT partition-crossing copy first
                        recs = aop.tile([1, 512], DT, tag="recs")
                        nc.scalar.activation(
                            recs[0:1, :], psoh[64:65, :], AF.Copy
                        )
                        nc.vector.reciprocal_approx_fast(
                            rec[0:1, 0:512], psoh[0:1, :]
                        )
                        nc.vector.reciprocal_approx_fast(
                            rec[0:1, 512:1024], recs[0:1, :]
                        )
                    else:
                        nc.vector.reciprocal(rec[0:1, 0:512], psoh[0:1, :])
                        nc.vector.reciprocal(rec[0:1, 512:1024], psoh[64:65, :])
                    denb = aop.tile([P, 512], DT, tag="denb")
                    nc.gpsimd.partition_broadcast(denb[:, :], rec[0:1, 512:1024])
                    nc.gpsimd.partition_broadcast(denb[0:64, :], rec[0:1, 0:512])
                    nc.vector.tensor_mul(
                        ao[:, 512 * h2 : 512 * h2 + 512], psoh[:], denb[:]
                    )

                # output projection
                for tcx in range(2):
                    psy = psY.tile([P, 384], DT, tag="psY")
                    for cc in range(4):
                        nc.tensor.matmul(
                            psy[:],
                            ao[:, 256 * cc + 128 * tcx : 256 * cc + 128 * tcx + 128],
                            wp_sb[cc][:],
                            start=(cc == 0),
                            stop=(cc == 3),
                        )
                    ysb = yp.tile([P, 384], DT, tag="y")
                    nc.vector.tensor_add(ysb[:], psy[:], bp_sb[:])
                    nc.sync.dma_start(
                        y_d[b, 128 * tcx : 128 * tcx + 128, :], ysb[:]
                    )

        if timing:
            nc.sync.dma_start(ydum_d[:], bp_sb[:, 0:4])

    nc.compile()
    return nc


def make_consts(attn_w, attn_b, proj_w, proj_b):
    attn_w = np.asarray(attn_w, dtype=np.float32)
    attn_b = np.asarray(attn_b, dtype=np.float32)
    proj_w = np.asarray(proj_w, dtype=np.float32)
    proj_b = np.asarray(proj_b, dtype=np.float32)

    s = 1.0 / np.sqrt(HD)
    Wq, Wk, Wv = attn_w[0:C], attn_w[C : 2 * C], attn_w[2 * C : 3 * C]
    bq, bk, bv = attn_b[0:C], attn_b[C : 2 * C], attn_b[2 * C : 3 * C]

    # WQK: [C, 1024] -> [3, 128, 1024]
    M = np.zeros((C, 1024), dtype=np.float32)
    for h in range(NH):
        Wq_h = Wq[HD * h : HD * h + HD]  # [48, C]
        Wk_h = Wk[HD * h : HD * h + HD]
        bq_h = bq[HD * h : HD * h + HD]
        bk_h = bk[HD * h : HD * h + HD]
        # q-hat block
        M[:, HB * h : HB * h + HD] = (s * Wq_h).T
        M[:, HB * h + 48] = s * (bk_h @ Wq_h)  # c_q row
        # (row 49 of q-hat is the ones row via bias)
        # k-hat block
        M[:, 512 + HB * h : 512 + HB * h + HD] = Wk_h.T
        # (row 48 of k-hat is the ones row via bias)
        M[:, 512 + HB * h + 49] = s * (bq_h @ Wk_h)  # c_k row
    WQK = np.ascontiguousarray(M.reshape(C, 1024).reshape(3, P, 1024))

    # WV: [C, 512] -> [3, 128, 512]; col HB*h+j (j<48) = Wv row HD*h+j
    V = np.zeros((C, 512), dtype=np.float32)
    for h in range(NH):
        V[:, HB * h : HB * h + HD] = Wv[HD * h : HD * h + HD].T
    WV = np.ascontiguousarray(V.reshape(3, P, 512))

    # WP: [512, 384] -> [4, 128, 384]; row HB*h + 1 + j = proj_w[:, HD*h+j]
    Wp_aug = np.zeros((512, C), dtype=np.float32)
    for h in range(NH):
        Wp_aug[HB * h + 1 : HB * h + 1 + HD, :] = proj_w[:, HD * h : HD * h + HD].T
    WP = np.ascontiguousarray(Wp_aug.reshape(4, P, 384))

    BQ = np.zeros((P, 1), dtype=np.float32)
    BQ[49, 0] = 1.0
    BQ[49 + HB, 0] = 1.0
    BK = np.zeros((P, 1), dtype=np.float32)
    BK[48, 0] = 1.0
    BK[48 + HB, 0] = 1.0

    # causal 0/1 mask for S^T[k, q] diagonal tiles (repeated x2 for head pairs)
    kk = np.arange(128)[:, None]
    qq = np.arange(128)[None, :]
    tri = (qq >= kk).astype(np.float32)  # [128k, 128q]
    TRI0 = np.ascontiguousarray(np.concatenate([tri, tri], axis=1)).astype(BF16_NP)

    bp_eff = proj_b + proj_w @ bv
    BP = np.ascontiguousarray(np.broadcast_to(bp_eff[None, :], (P, 384))).astype(
        np.float32
    )

    # v-init pattern: ones column at 64h, EPS_PAD at cols 49..63 of each block
    vinit_row = np.zeros(1024, dtype=np.float32)
    for kx in range(2):
        for h in range(NH):
            off = 512 * kx + HB * h
            vinit_row[off] = 1.0
            vinit_row[off + 49 : off + HB] = EPS_PAD
    VINIT = np.ascontiguousarray(np.broadcast_to(vinit_row[None, :], (P, 1024))).astype(
        BF16_NP
    )

    mmnp = BF16_NP if MM_BF16 else np.float32
    WQK = WQK.astype(mmnp)
    WV = WV.astype(mmnp)
    WP = WP.astype(mmnp)

    return {
        "vinit": VINIT,
        "wqk": WQK,
        "wv": WV,
        "wp": WP,
        "bq": BQ,
        "bk": BK,
        "tri0": TRI0,
        "bp": BP,
    }


_NC_CACHE = {}


def get_nc(nb: int = NB):
    if nb not in _NC_CACHE:
        _NC_CACHE[nb] = build_nc(nb)
    return _NC_CACHE[nb]


def make_in_maps(x, attn_w, attn_b, proj_w, proj_b):
    x = np.asarray(x, dtype=np.float32)
    consts = make_consts(attn_w, attn_b, proj_w, proj_b)
    in_maps = []
    for core in range(N_CORES):
        xs = x[core * NB : (core + 1) * NB]  # [NB, T, C]
        xTl = np.ascontiguousarray(xs.transpose(0, 2, 1))  # [NB, C, T]
        if MM_BF16:
            xTl = xTl.astype(BF16_NP)
        m = {"xT": xTl}
        m.update(consts)
        in_maps.append(m)
    return in_maps


def kernel(x, attn_w, attn_b, proj_w, proj_b):
    nc = get_nc(NB)
    in_maps = make_in_maps(x, attn_w, attn_b, proj_w, proj_b)
    res = run_bass_kernel_spmd(nc, in_maps, core_ids=list(range(N_CORES)))
    out = np.concatenate(
        [res.results[i]["y"] for i in range(N_CORES)], axis=0
    ).astype(np.float32)
    return out


# revision 13
# speedup vs baseline: 2.9262x; 1.0458x over previous
"""Causal self-attention Trainium2 kernel (B=256, T=256, C=384, 8 heads x 48).

Strategy: pure data-parallel over batch across 8 NeuronCores (32 batches per
core, no collectives). All layouts are arranged on the host so the device
kernel never transposes anything:

  - x is sent transposed per batch: xT [nb, C, T].
  - QK projection computes q^T / k^T in "feature-major" layout [o', tokens]
    with heads padded to 64-row blocks, augmented with 2 extra contraction
    rows that carry the q/k bias cross terms, so scores come out exactly
    (up to a per-head constant, which softmax ignores).  K_contract = 50.
  - Scores are computed transposed, S^T[k, q], per head with 2-head row-tiled
    matmul concurrency (head dim 50 <= 64 rows).  Causal block-sparsity: the
    (k in 128..255, q in 0..127) quadrant is fully masked, so its matmul,
    exp, mask and PV contributions are skipped entirely.  Per j-half layout:
    [kx0 q0..255 | kx1 q128..255 | dead].
  - Softmax skips the max-subtraction (inputs are well-scaled gaussians),
    exp on ACT straight PSUM->SBUF (strided, skipping the dead region),
    causal mask applied multiplicatively to the two triangular 128-blocks
    (both use the same tri pattern) on DVE.
  - The PV matmul consumes V in token-major layout (computed directly by
    swapping stationary/moving operands - no transpose), augmented with a
    leading ones column per head so row 0 of each head block is the softmax
    denominator.  Col-tiled 2-head concurrency; output per half-batch into a
    single-bank [128, 512] PSUM tile (bufs=2) for pipelining.
  - Normalization per half-batch: strided 2-row reciprocal_approx_fast of
    the denominator rows (~5x faster than vector.reciprocal), GpSimd
    partition-broadcast, one fused multiply PSUM->SBUF.
  - Output projection consumes the attention output directly in its
    [c', token] layout; V-bias is folded into the projection bias on host.

Matmuls run in bf16 (fp32 PSUM accumulation).
"""

import os
import sys

import numpy as np

try:
    import ml_dtypes

    BF16_NP = ml_dtypes.bfloat16
except ImportError:  # pragma: no cover
    BF16_NP = None

for _p in ("/opt/trn_rl_repo",):
    if os.path.isdir(_p) and _p not in sys.path:
        sys.path.insert(0, _p)

from contextlib import ExitStack

import concourse.bass as bass
import concourse.bacc as bacc
import concourse.tile as tile
from concourse import mybir
from concourse.bass_utils import run_bass_kernel_spmd

P = 128
T = 256
C = 384
NH = 8
HD = 48
KA = 50  # augmented contraction rows per head (48 + cq/ck row + ones row)
HB = 64  # padded head block stride
DT = mybir.dt.float32
DTR = mybir.dt.float32r
BF = mybir.dt.bfloat16
AF = mybir.ActivationFunctionType
N_CORES = 8
B_FULL = 256
NB = B_FULL // N_CORES  # batches per core

EPS_PAD = 1e-20  # value for padded V columns (keeps reciprocal finite)

MM_BF16 = os.environ.get("KERNEL_MM_BF16", "1") == "1"
MMDT = BF if MM_BF16 else DTR
FAST_RECIP = os.environ.get("KERNEL_FAST_RECIP", "1") == "1"


def build_nc(nb: int = NB, debug: bool = False, repeat: int = 1, timing: bool = False):
    nc = bacc.Bacc(None)

    xT = nc.declare_dram_parameter("xT", [nb, C, T], MMDT, isOutput=False)
    wqk_d = nc.declare_dram_parameter("wqk", [3, P, 1024], MMDT, isOutput=False)
    wv_d = nc.declare_dram_parameter("wv", [3, P, 384], MMDT, isOutput=False)
    wp_d = nc.declare_dram_parameter("wp", [4, P, 384], MMDT, isOutput=False)
    bq_d = nc.declare_dram_parameter("bq", [P, 1], DT, isOutput=False)
    bk_d = nc.declare_dram_parameter("bk", [P, 1], DT, isOutput=False)
    tri0_d = nc.declare_dram_parameter("tri0", [P, 256], BF, isOutput=False)
    bp_d = nc.declare_dram_parameter("bp", [P, 384], DT, isOutput=False)
    vinit_d = nc.declare_dram_parameter("vinit", [P, 1024], BF, isOutput=False)
    if timing:
        y_d = nc.dram_tensor("y_int", [nb, T, C], DT)
        ydum_d = nc.declare_dram_parameter("ydum", [P, 4], DT, isOutput=True)
    else:
        y_d = nc.declare_dram_parameter("y", [nb, T, C], DT, isOutput=True)

    with tile.TileContext(nc) as tc, ExitStack() as ctx:
        const = ctx.enter_context(tc.tile_pool(name="const", bufs=1))
        xtp = ctx.enter_context(tc.tile_pool(name="xt", bufs=6))
        qkp = ctx.enter_context(tc.tile_pool(name="qkt", bufs=2))
        vp = ctx.enter_context(tc.tile_pool(name="v", bufs=2))
        ptp = ctx.enter_context(tc.tile_pool(name="pt", bufs=2))
        aop = ctx.enter_context(tc.tile_pool(name="ao", bufs=2))
        yp = ctx.enter_context(tc.tile_pool(name="y", bufs=4))
        psA = ctx.enter_context(
            tc.tile_pool(name="psA", bufs=2, space=bass.MemorySpace.PSUM)
        )
        psS = ctx.enter_context(
            tc.tile_pool(name="psS", bufs=1, space=bass.MemorySpace.PSUM)
        )
        psO = ctx.enter_context(
            tc.tile_pool(name="psO", bufs=2, space=bass.MemorySpace.PSUM)
        )
        psY = ctx.enter_context(
            tc.tile_pool(name="psY", bufs=2, space=bass.MemorySpace.PSUM)
        )

        # ---- load constants ------------------------------------------------
        wqk_sb = []
        wv_sb = []
        wp_sb = []
        for ci in range(3):
            t = const.tile([P, 1024], MMDT, tag=f"wqk{ci}")
            nc.sync.dma_start(t[:], wqk_d[ci])
            wqk_sb.append(t)
        for ci in range(3):
            t = const.tile([P, 384], MMDT, tag=f"wv{ci}")
            nc.sync.dma_start(t[:], wv_d[ci])
            wv_sb.append(t)
        for cc in range(4):
            t = const.tile([P, 384], MMDT, tag=f"wp{cc}")
            nc.sync.dma_start(t[:], wp_d[cc])
            wp_sb.append(t)
        bq_sb = const.tile([P, 1], DT, tag="bq")
        nc.sync.dma_start(bq_sb[:], bq_d[:])
        bk_sb = const.tile([P, 1], DT, tag="bk")
        nc.sync.dma_start(bk_sb[:], bk_d[:])
        tri0_sb = const.tile([P, 256], BF, tag="tri0")
        nc.sync.dma_start(tri0_sb[:], tri0_d[:])
        bp_sb = const.tile([P, 384], DT, tag="bp")
        nc.sync.dma_start(bp_sb[:], bp_d[:])

        tri0_r = tri0_sb[:].rearrange("p (j r) -> p j r", r=128)

        # ---- per-batch-pair pipeline ---------------------------------------
        assert nb % 2 == 0
        for bp_it in range((nb // 2) * repeat):
            b0 = 2 * (bp_it % (nb // 2))
            xt = []
            for ci in range(3):
                t = xtp.tile([P, 2 * T], MMDT, tag="xt")
                nc.sync.dma_start(
                    t[:].rearrange("p (b t) -> p b t", b=2),
                    xT[b0 : b0 + 2, 128 * ci : 128 * ci + 128, :].rearrange(
                        "b p t -> p b t"
                    ),
                )
                xt.append(t)

            # QK projection: 2 waves x 4 single-bank [128, 512] psum tiles
            # (bufs=2 so next chunk's matmuls overlap this chunk's bias-move)
            qk_sb = {}
            for w, name in ((0, "qt"), (1, "kt")):
                dst = qkp.tile([P, 2048], MMDT, tag=name)
                bias = bq_sb if w == 0 else bk_sb
                for oc in range(4):
                    ps = psA.tile([P, 512], DT, tag="psA")
                    for ci in range(3):
                        nc.tensor.matmul(
                            ps[:],
                            wqk_sb[ci][
                                :, 512 * w + 128 * oc : 512 * w + 128 * oc + 128
                            ],
                            xt[ci][:],
                            start=(ci == 0),
                            stop=(ci == 2),
                        )
                    dst_sl = dst[:, 512 * oc : 512 * oc + 512]
                    if (w + oc) % 2 == 0:
                        nc.vector.tensor_scalar_add(dst_sl, ps[:], bias[:, 0:1])
                    else:
                        nc.scalar.activation(
                            dst_sl, ps[:], AF.Identity, bias=bias[:, 0:1]
                        )
                qk_sb[name] = dst
            qt, kt = qk_sb["qt"], qk_sb["kt"]

            # V in token-major layout (per batch), ones column + eps pads
            v_sbs = []
            for bb in range(2):
                vt = vp.tile([P, 1024], BF, tag="v")
                nc.sync.dma_start(vt[:], vinit_d[:])
                v_sbs.append(vt)
            for tch in range(4):
                bb, tcx = tch // 2, tch % 2
                psv = psA.tile([P, 512], DT, tag="psA")
                for ci in range(3):
                    nc.tensor.matmul(
                        psv[:, 0:384],
                        xt[ci][:, 256 * bb + 128 * tcx : 256 * bb + 128 * tcx + 128],
                        wv_sb[ci][:],
                        start=(ci == 0),
                        stop=(ci == 2),
                    )
                half = v_sbs[bb][:, 512 * tcx : 512 * tcx + 512].rearrange(
                    "p (h c) -> p h c", c=HB
                )
                psv_r = psv[:, 0:384].rearrange("p (h c) -> p h c", c=48)
                nc.scalar.activation(half[:, :, 1:49], psv_r[:, :, :], AF.Copy)

            for bb in range(2):
                b = b0 + bb
                v_sb = v_sbs[bb]
                # S^T per head pair + exp + mask + PV, per half-batch psO
                pt = ptp.tile([P, 4096], BF, tag="pt")
                ao = aop.tile([P, 1024], MMDT, tag="ao")
                for h2 in range(2):
                    psoh = psO.tile([P, 512], DT, tag="psO")
                    for gg in range(2):
                        g = 2 * h2 + gg
                        pss = psS.tile([P, 1024], DT, tag="psS")
                        qb = 512 * g + 256 * bb
                        for j in range(2):
                            base = HB * j
                            # kx0: keys 0..127, all 256 queries
                            nc.tensor.matmul(
                                pss[:, 512 * j : 512 * j + 256],
                                kt[base : base + KA, qb : qb + 128],
                                qt[base : base + KA, qb : qb + 256],
                                start=True,
                                stop=True,
                            )
                            # kx1: keys 128..255, queries 128..255 only
                            # (q<128 is fully causal-masked for these keys)
                            nc.tensor.matmul(
                                pss[:, 512 * j + 256 : 512 * j + 384],
                                kt[base : base + KA, qb + 128 : qb + 256],
                                qt[base : base + KA, qb + 128 : qb + 256],
                                start=True,
                                stop=True,
                            )
                        pt_g = pt[:, 1024 * g : 1024 * g + 1024].rearrange(
                            "p (j r) -> p j r", r=512
                        )
                        pss_r = pss[:].rearrange("p (j r) -> p j r", r=512)
                        nc.scalar.activation(
                            pt_g[:, :, 0:384], pss_r[:, :, 0:384], AF.Exp
                        )
                        # triangular masks: diagonal blocks of kx0 (cols
                        # 0..127) and kx1 (cols 256..383) share the pattern
                        nc.vector.tensor_mul(
                            pt_g[:, :, 0:128], pt_g[:, :, 0:128], tri0_r
                        )
                        nc.vector.tensor_mul(
                            pt_g[:, :, 256:384], pt_g[:, :, 256:384], tri0_r
                        )
                        # PV for this group
                        for j in range(2):
                            h = 2 * g + j
                            pc = 1024 * g + 512 * j
                            nc.tensor.matmul(
                                psoh[HB * j : HB * j + HB, 256 * gg : 256 * gg + 256],
                                v_sb[:, HB * h : HB * h + HB],
                                pt[:, pc : pc + 256],
                                start=True,
                                stop=False,
                                tile_position=(0, HB * j),
                            )
                            nc.tensor.matmul(
                                psoh[
                                    HB * j : HB * j + HB,
                                    256 * gg + 128 : 256 * gg + 256,
                                ],
                                v_sb[:, 512 + HB * h : 512 + HB * h + HB],
                                pt[:, pc + 256 : pc + 384],
                                start=False,
                                stop=True,
                                tile_position=(0, HB * j),
                            )

                    # normalize this half-batch: rows 0 / 64 hold denominators
                    rec = aop.tile([1, 1024], DT, tag="rec")
                    if FAST_RECIP:
                        # custom-DVE op only works with partition-0 base on
                        # HW; stage the j1 den row (partition 64) to p0 via
                        # an ACT partition-crossing copy first
                        recs = aop.tile([1, 512], DT, tag="recs")
                        nc.scalar.activation(
                            recs[0:1, :], psoh[64:65, :], AF.Copy
                        )
                        nc.vector.reciprocal_approx_fast(
                            rec[0:1, 0:512], psoh[0:1, :]
                        )
                        nc.vector.reciprocal_approx_fast(
                            rec[0:1, 512:1024], recs[0:1, :]
                        )
                    else:
                        nc.vector.reciprocal(rec[0:1, 0:512], psoh[0:1, :])
                        nc.vector.reciprocal(rec[0:1, 512:1024], psoh[64:65, :])
                    denb = aop.tile([P, 512], DT, tag="denb")
                    nc.gpsimd.partition_broadcast(denb[:, :], rec[0:1, 512:1024])
                    nc.gpsimd.partition_broadcast(denb[0:64, :], rec[0:1, 0:512])
                    nc.vector.tensor_mul(
                        ao[:, 512 * h2 : 512 * h2 + 512], psoh[:], denb[:]
                    )

                # output projection
                for tcx in range(2):
                    psy = psY.tile([P, 384], DT, tag="psY")
                    for cc in range(4):
                        nc.tensor.matmul(
                            psy[:],
                            ao[:, 256 * cc + 128 * tcx : 256 * cc + 128 * tcx + 128],
                            wp_sb[cc][:],
                            start=(cc == 0),
                            stop=(cc == 3),
                        )
                    ysb = yp.tile([P, 384], DT, tag="y")
                    nc.vector.tensor_add(ysb[:], psy[:], bp_sb[:])
                    nc.sync.dma_start(
                        y_d[b, 128 * tcx : 128 * tcx + 128, :], ysb[:]
                    )

        if timing:
            nc.sync.dma_start(ydum_d[:], bp_sb[:, 0:4])

    nc.compile()
    return nc


def make_consts(attn_w, attn_b, proj_w, proj_b):
    attn_w = np.asarray(attn_w, dtype=np.float32)
    attn_b = np.asarray(attn_b, dtype=np.float32)
    proj_w = np.asarray(proj_w, dtype=np.float32)
    proj_b = np.asarray(proj_b, dtype=np.float32)

    s = 1.0 / np.sqrt(HD)
    Wq, Wk, Wv = attn_w[0:C], attn_w[C : 2 * C], attn_w[2 * C : 3 * C]
    bq, bk, bv = attn_b[0:C], attn_b[C : 2 * C], attn_b[2 * C : 3 * C]

    # WQK: [C, 1024] -> [3, 128, 1024]
    M = np.zeros((C, 1024), dtype=np.float32)
    for h in range(NH):
        Wq_h = Wq[HD * h : HD * h + HD]  # [48, C]
        Wk_h = Wk[HD * h : HD * h + HD]
        bq_h = bq[HD * h : HD * h + HD]
        bk_h = bk[HD * h : HD * h + HD]
        # q-hat block
        M[:, HB * h : HB * h + HD] = (s * Wq_h).T
        M[:, HB * h + 48] = s * (bk_h @ Wq_h)  # c_q row
        # (row 49 of q-hat is the ones row via bias)
        # k-hat block
        M[:, 512 + HB * h : 512 + HB * h + HD] = Wk_h.T
        # (row 48 of k-hat is the ones row via bias)
        M[:, 512 + HB * h + 49] = s * (bq_h @ Wk_h)  # c_k row
    WQK = np.ascontiguousarray(M.reshape(C, 1024).reshape(3, P, 1024))

    # WV: [C, 384] -> [3, 128, 384]; col HD*h+j = Wv row HD*h+j (packed)
    V = np.zeros((C, 384), dtype=np.float32)
    for h in range(NH):
        V[:, HD * h : HD * h + HD] = Wv[HD * h : HD * h + HD].T
    WV = np.ascontiguousarray(V.reshape(3, P, 384))

    # WP: [512, 384] -> [4, 128, 384]; row HB*h + 1 + j = proj_w[:, HD*h+j]
    Wp_aug = np.zeros((512, C), dtype=np.float32)
    for h in range(NH):
        Wp_aug[HB * h + 1 : HB * h + 1 + HD, :] = proj_w[:, HD * h : HD * h + HD].T
    WP = np.ascontiguousarray(Wp_aug.reshape(4, P, 384))

    BQ = np.zeros((P, 1), dtype=np.float32)
    BQ[49, 0] = 1.0
    BQ[49 + HB, 0] = 1.0
    BK = np.zeros((P, 1), dtype=np.float32)
    BK[48, 0] = 1.0
    BK[48 + HB, 0] = 1.0

    # causal 0/1 mask for S^T[k, q] diagonal tiles (repeated x2 for head pairs)
    kk = np.arange(128)[:, None]
    qq = np.arange(128)[None, :]
    tri = (qq >= kk).astype(np.float32)  # [128k, 128q]
    TRI0 = np.ascontiguousarray(np.concatenate([tri, tri], axis=1)).astype(BF16_NP)

    bp_eff = proj_b + proj_w @ bv
    BP = np.ascontiguousarray(np.broadcast_to(bp_eff[None, :], (P, 384))).astype(
        np.float32
    )

    # v-init pattern: ones column at 64h, EPS_PAD at cols 49..63 of each block
    vinit_row = np.zeros(1024, dtype=np.float32)
    for kx in range(2):
        for h in range(NH):
            off = 512 * kx + HB * h
            vinit_row[off] = 1.0
            vinit_row[off + 49 : off + HB] = EPS_PAD
    VINIT = np.ascontiguousarray(np.broadcast_to(vinit_row[None, :], (P, 1024))).astype(
        BF16_NP
    )

    mmnp = BF16_NP if MM_BF16 else np.float32
    WQK = WQK.astype(mmnp)
    WV = WV.astype(mmnp)
    WP = WP.astype(mmnp)

    return {
        "vinit": VINIT,
        "wqk": WQK,
        "wv": WV,
        "wp": WP,
        "bq": BQ,
        "bk": BK,
        "tri0": TRI0,
        "bp": BP,
    }


_NC_CACHE = {}


def get_nc(nb: int = NB):
    if nb not in _NC_CACHE:
        _NC_CACHE[nb] = build_nc(nb)
    return _NC_CACHE[nb]


def make_in_maps(x, attn_w, attn_b, proj_w, proj_b):
    x = np.asarray(x, dtype=np.float32)
    consts = make_consts(attn_w, attn_b, proj_w, proj_b)
    in_maps = []
    for core in range(N_CORES):
        xs = x[core * NB : (core + 1) * NB]  # [NB, T, C]
        xTl = np.ascontiguousarray(xs.transpose(0, 2, 1))  # [NB, C, T]
        if MM_BF16:
            xTl = xTl.astype(BF16_NP)
        m = {"xT": xTl}
        m.update(consts)
        in_maps.append(m)
    return in_maps


def kernel(x, attn_w, attn_b, proj_w, proj_b):
    nc = get_nc(NB)
    in_maps = make_in_maps(x, attn_w, attn_b, proj_w, proj_b)
    res = run_bass_kernel_spmd(nc, in_maps, core_ids=list(range(N_CORES)))
    out = np.concatenate(
        [res.results[i]["y"] for i in range(N_CORES)], axis=0
    ).astype(np.float32)
    return out


# revision 16
# speedup vs baseline: 2.9372x; 1.0037x over previous
"""Causal self-attention Trainium2 kernel (B=256, T=256, C=384, 8 heads x 48).

Strategy: pure data-parallel over batch across 8 NeuronCores (32 batches per
core, no collectives). All layouts are arranged on the host so the device
kernel never transposes anything:

  - x is sent transposed per batch: xT [nb, C, T].
  - QK projection computes q^T / k^T in "feature-major" layout [o', tokens]
    with heads padded to 64-row blocks, augmented with 2 extra contraction
    rows that carry the q/k bias cross terms, so scores come out exactly
    (up to a per-head constant, which softmax ignores).  K_contract = 50.
  - Scores are computed transposed, S^T[k, q], per head with 2-head row-tiled
    matmul concurrency (head dim 50 <= 64 rows).  Causal block-sparsity: the
    (k in 128..255, q in 0..127) quadrant is fully masked, so its matmul,
    exp, mask and PV contributions are skipped entirely.  Per j-half layout:
    [kx0 q0..255 | kx1 q128..255 | dead].
  - Softmax skips the max-subtraction (inputs are well-scaled gaussians),
    exp on ACT straight PSUM->SBUF (strided, skipping the dead region),
    causal mask applied multiplicatively to the two triangular 128-blocks
    (both use the same tri pattern) on DVE.
  - The PV matmul consumes V in token-major layout (computed directly by
    swapping stationary/moving operands - no transpose), augmented with a
    leading ones column per head so row 0 of each head block is the softmax
    denominator.  Col-tiled 2-head concurrency; output per half-batch into a
    single-bank [128, 512] PSUM tile (bufs=2) for pipelining.
  - Normalization per half-batch: strided 2-row reciprocal_approx_fast of
    the denominator rows (~5x faster than vector.reciprocal), GpSimd
    partition-broadcast, one fused multiply PSUM->SBUF.
  - Output projection consumes the attention output directly in its
    [c', token] layout; V-bias is folded into the projection bias on host.

Matmuls run in bf16 (fp32 PSUM accumulation).
"""

import os
import sys

import numpy as np

try:
    import ml_dtypes

    BF16_NP = ml_dtypes.bfloat16
except ImportError:  # pragma: no cover
    BF16_NP = None

for _p in ("/opt/trn_rl_repo",):
    if os.path.isdir(_p) and _p not in sys.path:
        sys.path.insert(0, _p)

from contextlib import ExitStack

import concourse.bass as bass
import concourse.bacc as bacc
import concourse.tile as tile
from concourse import mybir
from concourse.bass_utils import run_bass_kernel_spmd

P = 128
T = 256
C = 384
NH = 8
HD = 48
KA = 50  # augmented contraction rows per head (48 + cq/ck row + ones row)
HB = 64  # padded head block stride
DT = mybir.dt.float32
DTR = mybir.dt.float32r
BF = mybir.dt.bfloat16
AF = mybir.ActivationFunctionType
N_CORES = 8
B_FULL = 256
NB = B_FULL // N_CORES  # batches per core

EPS_PAD = 1e-20  # value for padded V columns (keeps reciprocal finite)

MM_BF16 = os.environ.get("KERNEL_MM_BF16", "1") == "1"
MMDT = BF if MM_BF16 else DTR
FAST_RECIP = os.environ.get("KERNEL_FAST_RECIP", "1") == "1"


def build_nc(nb: int = NB, debug: bool = False, repeat: int = 1, timing: bool = False):
    nc = bacc.Bacc(None)

    xT = nc.declare_dram_parameter("xT", [nb, C, T], MMDT, isOutput=False)
    wqk_d = nc.declare_dram_parameter("wqk", [3, P, 1024], MMDT, isOutput=False)
    wv_d = nc.declare_dram_parameter("wv", [3, P, 384], MMDT, isOutput=False)
    wp_d = nc.declare_dram_parameter("wp", [4, P, 384], MMDT, isOutput=False)
    bq_d = nc.declare_dram_parameter("bq", [P, 1], DT, isOutput=False)
    bk_d = nc.declare_dram_parameter("bk", [P, 1], DT, isOutput=False)
    tri0_d = nc.declare_dram_parameter("tri0", [P, 256], BF, isOutput=False)
    bp_d = nc.declare_dram_parameter("bp", [P, 384], DT, isOutput=False)
    vinit_d = nc.declare_dram_parameter("vinit", [P, 1024], BF, isOutput=False)
    if timing:
        y_d = nc.dram_tensor("y_int", [nb, T, C], DT)
        ydum_d = nc.declare_dram_parameter("ydum", [P, 4], DT, isOutput=True)
    else:
        y_d = nc.declare_dram_parameter("y", [nb, T, C], DT, isOutput=True)

    with tile.TileContext(nc) as tc, ExitStack() as ctx:
        const = ctx.enter_context(tc.tile_pool(name="const", bufs=1))
        xtp = ctx.enter_context(tc.tile_pool(name="xt", bufs=6))
        qkp = ctx.enter_context(tc.tile_pool(name="qkt", bufs=2))
        vp = ctx.enter_context(tc.tile_pool(name="v", bufs=2))
        ptp = ctx.enter_context(tc.tile_pool(name="pt", bufs=2))
        aop = ctx.enter_context(tc.tile_pool(name="ao", bufs=2))
        yp = ctx.enter_context(tc.tile_pool(name="y", bufs=4))
        psA = ctx.enter_context(
            tc.tile_pool(name="psA", bufs=2, space=bass.MemorySpace.PSUM)
        )
        psS = ctx.enter_context(
            tc.tile_pool(name="psS", bufs=1, space=bass.MemorySpace.PSUM)
        )
        psO = ctx.enter_context(
            tc.tile_pool(name="psO", bufs=2, space=bass.MemorySpace.PSUM)
        )
        psY = ctx.enter_context(
            tc.tile_pool(name="psY", bufs=2, space=bass.MemorySpace.PSUM)
        )

        # ---- load constants ------------------------------------------------
        wqk_sb = []
        wv_sb = []
        wp_sb = []
        for ci in range(3):
            t = const.tile([P, 1024], MMDT, tag=f"wqk{ci}")
            nc.sync.dma_start(t[:], wqk_d[ci])
            wqk_sb.append(t)
        for ci in range(3):
            t = const.tile([P, 384], MMDT, tag=f"wv{ci}")
            nc.sync.dma_start(t[:], wv_d[ci])
            wv_sb.append(t)
        for cc in range(4):
            t = const.tile([P, 384], MMDT, tag=f"wp{cc}")
            nc.sync.dma_start(t[:], wp_d[cc])
            wp_sb.append(t)
        bq_sb = const.tile([P, 1], DT, tag="bq")
        nc.sync.dma_start(bq_sb[:], bq_d[:])
        bk_sb = const.tile([P, 1], DT, tag="bk")
        nc.sync.dma_start(bk_sb[:], bk_d[:])
        tri0_sb = const.tile([P, 256], BF, tag="tri0")
        nc.sync.dma_start(tri0_sb[:], tri0_d[:])
        bp_sb = const.tile([P, 384], DT, tag="bp")
        nc.sync.dma_start(bp_sb[:], bp_d[:])

        tri0_r = tri0_sb[:].rearrange("p (j r) -> p j r", r=128)

        # ---- per-batch-pair pipeline ---------------------------------------
        assert nb % 2 == 0

        def emit_proj(b, ao):
            for tcx in range(2):
                psy = psY.tile([P, 384], DT, tag="psY")
                for cc in range(4):
                    nc.tensor.matmul(
                        psy[:],
                        ao[:, 256 * cc + 128 * tcx : 256 * cc + 128 * tcx + 128],
                        wp_sb[cc][:],
                        start=(cc == 0),
                        stop=(cc == 3),
                    )
                ysb = yp.tile([P, 384], DT, tag="y")
                nc.vector.tensor_add(ysb[:], psy[:], bp_sb[:])
                nc.sync.dma_start(y_d[b, 128 * tcx : 128 * tcx + 128, :], ysb[:])

        pending_proj = None
        for bp_it in range((nb // 2) * repeat):
            b0 = 2 * (bp_it % (nb // 2))
            xt = []
            for ci in range(3):
                t = xtp.tile([P, 2 * T], MMDT, tag="xt")
                nc.sync.dma_start(
                    t[:].rearrange("p (b t) -> p b t", b=2),
                    xT[b0 : b0 + 2, 128 * ci : 128 * ci + 128, :].rearrange(
                        "b p t -> p b t"
                    ),
                )
                xt.append(t)

            # QK projection: 2 waves x 4 single-bank [128, 512] psum tiles
            # (bufs=2 so next chunk's matmuls overlap this chunk's bias-move)
            qk_sb = {}
            for w, name in ((0, "qt"), (1, "kt")):
                dst = qkp.tile([P, 2048], MMDT, tag=name)
                bias = bq_sb if w == 0 else bk_sb
                for oc in range(4):
                    ps = psA.tile([P, 512], DT, tag="psA")
                    for ci in range(3):
                        nc.tensor.matmul(
                            ps[:],
                            wqk_sb[ci][
                                :, 512 * w + 128 * oc : 512 * w + 128 * oc + 128
                            ],
                            xt[ci][:],
                            start=(ci == 0),
                            stop=(ci == 2),
                        )
                    dst_sl = dst[:, 512 * oc : 512 * oc + 512]
                    if (w + oc) % 2 == 0:
                        nc.vector.tensor_scalar_add(dst_sl, ps[:], bias[:, 0:1])
                    else:
                        nc.scalar.activation(
                            dst_sl, ps[:], AF.Identity, bias=bias[:, 0:1]
                        )
                qk_sb[name] = dst
            qt, kt = qk_sb["qt"], qk_sb["kt"]

            # V in token-major layout (per batch), ones column + eps pads
            v_sbs = []
            for bb in range(2):
                vt = vp.tile([P, 1024], BF, tag="v")
                nc.sync.dma_start(vt[:], vinit_d[:])
                v_sbs.append(vt)
            for tch in range(4):
                bb, tcx = tch // 2, tch % 2
                psv = psA.tile([P, 512], DT, tag="psA")
                for ci in range(3):
                    nc.tensor.matmul(
                        psv[:, 0:384],
                        xt[ci][:, 256 * bb + 128 * tcx : 256 * bb + 128 * tcx + 128],
                        wv_sb[ci][:],
                        start=(ci == 0),
                        stop=(ci == 2),
                    )
                half = v_sbs[bb][:, 512 * tcx : 512 * tcx + 512].rearrange(
                    "p (h c) -> p h c", c=HB
                )
                psv_r = psv[:, 0:384].rearrange("p (h c) -> p h c", c=48)
                nc.scalar.activation(half[:, :, 1:49], psv_r[:, :, :], AF.Copy)

            for bb in range(2):
                b = b0 + bb
                v_sb = v_sbs[bb]
                # S^T per head pair + exp + mask + PV, per half-batch psO
                pt = ptp.tile([P, 4096], BF, tag="pt")
                ao = aop.tile([P, 1024], MMDT, tag="ao")
                for h2 in range(2):
                    psoh = psO.tile([P, 512], DT, tag="psO")
                    for gg in range(2):
                        g = 2 * h2 + gg
                        if g == 1 and pending_proj is not None:
                            # flush the previous batch's deferred projection
                            # here so its matmuls queue behind this batch's
                            # first attention group (keeps PE stream dense)
                            emit_proj(*pending_proj)
                            pending_proj = None
                        pss = psS.tile([P, 1024], DT, tag="psS")
                        qb = 512 * g + 256 * bb
                        for j in range(2):
                            base = HB * j
                            # kx0: keys 0..127, all 256 queries
                            nc.tensor.matmul(
                                pss[:, 512 * j : 512 * j + 256],
                                kt[base : base + KA, qb : qb + 128],
                                qt[base : base + KA, qb : qb + 256],
                                start=True,
                                stop=True,
                            )
                            # kx1: keys 128..255, queries 128..255 only
                            # (q<128 is fully causal-masked for these keys)
                            nc.tensor.matmul(
                                pss[:, 512 * j + 256 : 512 * j + 384],
                                kt[base : base + KA, qb + 128 : qb + 256],
                                qt[base : base + KA, qb + 128 : qb + 256],
                                start=True,
                                stop=True,
                            )
                        pt_g = pt[:, 1024 * g : 1024 * g + 1024].rearrange(
                            "p (j r) -> p j r", r=512
                        )
                        pss_r = pss[:].rearrange("p (j r) -> p j r", r=512)
                        nc.scalar.activation(
                            pt_g[:, :, 0:384], pss_r[:, :, 0:384], AF.Exp
                        )
                        # triangular masks: diagonal blocks of kx0 (cols
                        # 0..127) and kx1 (cols 256..383) share the pattern
                        nc.vector.tensor_mul(
                            pt_g[:, :, 0:128], pt_g[:, :, 0:128], tri0_r
                        )
                        nc.vector.tensor_mul(
                            pt_g[:, :, 256:384], pt_g[:, :, 256:384], tri0_r
                        )
                        # PV for this group
                        for j in range(2):
                            h = 2 * g + j
                            pc = 1024 * g + 512 * j
                            nc.tensor.matmul(
                                psoh[HB * j : HB * j + HB, 256 * gg : 256 * gg + 256],
                                v_sb[:, HB * h : HB * h + HB],
                                pt[:, pc : pc + 256],
                                start=True,
                                stop=False,
                                tile_position=(0, HB * j),
                            )
                            nc.tensor.matmul(
                                psoh[
                                    HB * j : HB * j + HB,
                                    256 * gg + 128 : 256 * gg + 256,
                                ],
                                v_sb[:, 512 + HB * h : 512 + HB * h + HB],
                                pt[:, pc + 256 : pc + 384],
                                start=False,
                                stop=True,
                                tile_position=(0, HB * j),
                            )

                    # normalize this half-batch: rows 0 / 64 hold denominators
                    rec = aop.tile([1, 1024], DT, tag="rec")
                    if FAST_RECIP:
                        # custom-DVE op only works with partition-0 base on
                        # HW; stage the j1 den row (partition 64) to p0 via
                        # an ACT partition-crossing copy first
                        recs = aop.tile([1, 512], DT, tag="recs")
                        nc.scalar.activation(
                            recs[0:1, :], psoh[64:65, :], AF.Copy
                        )
                        nc.vector.reciprocal_approx_fast(
                            rec[0:1, 0:512], psoh[0:1, :]
                        )
                        nc.vector.reciprocal_approx_fast(
                            rec[0:1, 512:1024], recs[0:1, :]
                        )
                    else:
                        nc.vector.reciprocal(rec[0:1, 0:512], psoh[0:1, :])
                        nc.vector.reciprocal(rec[0:1, 512:1024], psoh[64:65, :])
                    denb = aop.tile([P, 512], DT, tag="denb")
                    nc.gpsimd.partition_broadcast(denb[:, :], rec[0:1, 512:1024])
                    nc.gpsimd.partition_broadcast(denb[0:64, :], rec[0:1, 0:512])
                    nc.vector.tensor_mul(
                        ao[:, 512 * h2 : 512 * h2 + 512], psoh[:], denb[:]
                    )

                # output projection: deferred — emitted during the next
                # batch's attention so PE never waits on the normalize tail
                pending_proj = (b, ao)

        if pending_proj is not None:
            emit_proj(*pending_proj)
            pending_proj = None

        if timing:
            nc.sync.dma_start(ydum_d[:], bp_sb[:, 0:4])

    nc.compile()
    return nc


def make_consts(attn_w, attn_b, proj_w, proj_b):
    attn_w = np.asarray(attn_w, dtype=np.float32)
    attn_b = np.asarray(attn_b, dtype=np.float32)
    proj_w = np.asarray(proj_w, dtype=np.float32)
    proj_b = np.asarray(proj_b, dtype=np.float32)

    s = 1.0 / np.sqrt(HD)
    Wq, Wk, Wv = attn_w[0:C], attn_w[C : 2 * C], attn_w[2 * C : 3 * C]
    bq, bk, bv = attn_b[0:C], attn_b[C : 2 * C], attn_b[2 * C : 3 * C]

    # WQK: [C, 1024] -> [3, 128, 1024]
    M = np.zeros((C, 1024), dtype=np.float32)
    for h in range(NH):
        Wq_h = Wq[HD * h : HD * h + HD]  # [48, C]
        Wk_h = Wk[HD * h : HD * h + HD]
        bq_h = bq[HD * h : HD * h + HD]
        bk_h = bk[HD * h : HD * h + HD]
        # q-hat block
        M[:, HB * h : HB * h + HD] = (s * Wq_h).T
        M[:, HB * h + 48] = s * (bk_h @ Wq_h)  # c_q row
        # (row 49 of q-hat is the ones row via bias)
        # k-hat block
        M[:, 512 + HB * h : 512 + HB * h + HD] = Wk_h.T
        # (row 48 of k-hat is the ones row via bias)
        M[:, 512 + HB * h + 49] = s * (bq_h @ Wk_h)  # c_k row
    WQK = np.ascontiguousarray(M.reshape(C, 1024).reshape(3, P, 1024))

    # WV: [C, 384] -> [3, 128, 384]; col HD*h+j = Wv row HD*h+j (packed)
    V = np.zeros((C, 384), dtype=np.float32)
    for h in range(NH):
        V[:, HD * h : HD * h + HD] = Wv[HD * h : HD * h + HD].T
    WV = np.ascontiguousarray(V.reshape(3, P, 384))

    # WP: [512, 384] -> [4, 128, 384]; row HB*h + 1 + j = proj_w[:, HD*h+j]
    Wp_aug = np.zeros((512, C), dtype=np.float32)
    for h in range(NH):
        Wp_aug[HB * h + 1 : HB * h + 1 + HD, :] = proj_w[:, HD * h : HD * h + HD].T
    WP = np.ascontiguousarray(Wp_aug.reshape(4, P, 384))

    BQ = np.zeros((P, 1), dtype=np.float32)
    BQ[49, 0] = 1.0
    BQ[49 + HB, 0] = 1.0
    BK = np.zeros((P, 1), dtype=np.float32)
    BK[48, 0] = 1.0
    BK[48 + HB, 0] = 1.0

    # causal 0/1 mask for S^T[k, q] diagonal tiles (repeated x2 for head pairs)
    kk = np.arange(128)[:, None]
    qq = np.arange(128)[None, :]
    tri = (qq >= kk).astype(np.float32)  # [128k, 128q]
    TRI0 = np.ascontiguousarray(np.concatenate([tri, tri], axis=1)).astype(BF16_NP)

    bp_eff = proj_b + proj_w @ bv
    BP = np.ascontiguousarray(np.broadcast_to(bp_eff[None, :], (P, 384))).astype(
        np.float32
    )

    # v-init pattern: ones column at 64h, EPS_PAD at cols 49..63 of each block
    vinit_row = np.zeros(1024, dtype=np.float32)
    for kx in range(2):
        for h in range(NH):
            off = 512 * kx + HB * h
            vinit_row[off] = 1.0
            vinit_row[off + 49 : off + HB] = EPS_PAD
    VINIT = np.ascontiguousarray(np.broadcast_to(vinit_row[None, :], (P, 1024))).astype(
        BF16_NP
    )

    mmnp = BF16_NP if MM_BF16 else np.float32
    WQK = WQK.astype(mmnp)
    WV = WV.astype(mmnp)
    WP = WP.astype(mmnp)

    return {
        "vinit": VINIT,
        "wqk": WQK,
        "wv": WV,
        "wp": WP,
        "bq": BQ,
        "bk": BK,
        "tri0": TRI0,
        "bp": BP,
    }


_NC_CACHE = {}


def get_nc(nb: int = NB):
    if nb not in _NC_CACHE:
        _NC_CACHE[nb] = build_nc(nb)
    return _NC_CACHE[nb]


def make_in_maps(x, attn_w, attn_b, proj_w, proj_b):
    x = np.asarray(x, dtype=np.float32)
    consts = make_consts(attn_w, attn_b, proj_w, proj_b)
    in_maps = []
    for core in range(N_CORES):
        xs = x[core * NB : (core + 1) * NB]  # [NB, T, C]
        xTl = np.ascontiguousarray(xs.transpose(0, 2, 1))  # [NB, C, T]
        if MM_BF16:
            xTl = xTl.astype(BF16_NP)
        m = {"xT": xTl}
        m.update(consts)
        in_maps.append(m)
    return in_maps


def kernel(x, attn_w, attn_b, proj_w, proj_b):
    nc = get_nc(NB)
    in_maps = make_in_maps(x, attn_w, attn_b, proj_w, proj_b)
    res = run_bass_kernel_spmd(nc, in_maps, core_ids=list(range(N_CORES)))
    out = np.concatenate(
        [res.results[i]["y"] for i in range(N_CORES)], axis=0
    ).astype(np.float32)
    return out


# revision 17
# speedup vs baseline: 3.0307x; 1.0318x over previous
"""Causal self-attention Trainium2 kernel (B=256, T=256, C=384, 8 heads x 48).

Strategy: pure data-parallel over batch across 8 NeuronCores (32 batches per
core, no collectives). All layouts are arranged on the host so the device
kernel never transposes anything:

  - x is sent transposed per batch: xT [nb, C, T].
  - QK projection computes q^T / k^T in "feature-major" layout [o', tokens]
    with heads padded to 64-row blocks, augmented with 2 extra contraction
    rows that carry the q/k bias cross terms, so scores come out exactly
    (up to a per-head constant, which softmax ignores).  K_contract = 50.
  - Scores are computed transposed, S^T[k, q], per head with 2-head row-tiled
    matmul concurrency (head dim 50 <= 64 rows).  Causal block-sparsity: the
    (k in 128..255, q in 0..127) quadrant is fully masked, so its matmul,
    exp, mask and PV contributions are skipped entirely.  Per j-half layout:
    [kx0 q0..255 | kx1 q128..255 | dead].
  - Softmax skips the max-subtraction (inputs are well-scaled gaussians),
    exp on ACT straight PSUM->SBUF (strided, skipping the dead region),
    causal mask applied multiplicatively to the two triangular 128-blocks
    (both use the same tri pattern) on DVE.
  - The PV matmul consumes V in token-major layout (computed directly by
    swapping stationary/moving operands - no transpose), augmented with a
    leading ones column per head so row 0 of each head block is the softmax
    denominator.  Col-tiled 2-head concurrency; output per half-batch into a
    single-bank [128, 512] PSUM tile (bufs=2) for pipelining.
  - Normalization per half-batch: strided 2-row reciprocal_approx_fast of
    the denominator rows (~5x faster than vector.reciprocal), GpSimd
    partition-broadcast, one fused multiply PSUM->SBUF.
  - Output projection consumes the attention output directly in its
    [c', token] layout; V-bias is folded into the projection bias on host.

Matmuls run in bf16 (fp32 PSUM accumulation).
"""

import os
import sys

import numpy as np

try:
    import ml_dtypes

    BF16_NP = ml_dtypes.bfloat16
except ImportError:  # pragma: no cover
    BF16_NP = None

for _p in ("/opt/trn_rl_repo",):
    if os.path.isdir(_p) and _p not in sys.path:
        sys.path.insert(0, _p)

from contextlib import ExitStack

import concourse.bass as bass
import concourse.bacc as bacc
import concourse.tile as tile
from concourse import mybir
from concourse.bass_utils import run_bass_kernel_spmd

P = 128
T = 256
C = 384
NH = 8
HD = 48
KA = 50  # augmented contraction rows per head (48 + cq/ck row + ones row)
HB = 64  # padded head block stride
DT = mybir.dt.float32
DTR = mybir.dt.float32r
BF = mybir.dt.bfloat16
AF = mybir.ActivationFunctionType
N_CORES = 8
B_FULL = 256
NB = B_FULL // N_CORES  # batches per core

EPS_PAD = 1e-20  # value for padded V columns (keeps reciprocal finite)

MM_BF16 = os.environ.get("KERNEL_MM_BF16", "1") == "1"
MMDT = BF if MM_BF16 else DTR
FAST_RECIP = os.environ.get("KERNEL_FAST_RECIP", "1") == "1"


def build_nc(nb: int = NB, debug: bool = False, repeat: int = 1, timing: bool = False):
    nc = bacc.Bacc(None)

    xT = nc.declare_dram_parameter("xT", [nb, C, T], MMDT, isOutput=False)
    wqk_d = nc.declare_dram_parameter("wqk", [3, P, 1024], MMDT, isOutput=False)
    wv_d = nc.declare_dram_parameter("wv", [3, P, 384], MMDT, isOutput=False)
    wp_d = nc.declare_dram_parameter("wp", [4, P, 384], MMDT, isOutput=False)
    bq_d = nc.declare_dram_parameter("bq", [P, 1], DT, isOutput=False)
    bk_d = nc.declare_dram_parameter("bk", [P, 1], DT, isOutput=False)
    tri0_d = nc.declare_dram_parameter("tri0", [P, 256], BF, isOutput=False)
    bp_d = nc.declare_dram_parameter("bp", [P, 384], DT, isOutput=False)
    vinit_d = nc.declare_dram_parameter("vinit", [P, 1024], BF, isOutput=False)
    if timing:
        y_d = nc.dram_tensor("y_int", [nb, T, C], DT)
        ydum_d = nc.declare_dram_parameter("ydum", [P, 4], DT, isOutput=True)
    else:
        y_d = nc.declare_dram_parameter("y", [nb, T, C], DT, isOutput=True)

    with tile.TileContext(nc) as tc, ExitStack() as ctx:
        const = ctx.enter_context(tc.tile_pool(name="const", bufs=1))
        xtp = ctx.enter_context(tc.tile_pool(name="xt", bufs=9))
        qkp = ctx.enter_context(tc.tile_pool(name="qkt", bufs=2))
        vp = ctx.enter_context(tc.tile_pool(name="v", bufs=3))
        ptp = ctx.enter_context(tc.tile_pool(name="pt", bufs=2))
        aop = ctx.enter_context(tc.tile_pool(name="ao", bufs=3))
        yp = ctx.enter_context(tc.tile_pool(name="y", bufs=6))
        psA = ctx.enter_context(
            tc.tile_pool(name="psA", bufs=2, space=bass.MemorySpace.PSUM)
        )
        psS = ctx.enter_context(
            tc.tile_pool(name="psS", bufs=1, space=bass.MemorySpace.PSUM)
        )
        psO = ctx.enter_context(
            tc.tile_pool(name="psO", bufs=2, space=bass.MemorySpace.PSUM)
        )
        psY = ctx.enter_context(
            tc.tile_pool(name="psY", bufs=2, space=bass.MemorySpace.PSUM)
        )

        # ---- load constants ------------------------------------------------
        wqk_sb = []
        wv_sb = []
        wp_sb = []
        for ci in range(3):
            t = const.tile([P, 1024], MMDT, tag=f"wqk{ci}")
            nc.sync.dma_start(t[:], wqk_d[ci])
            wqk_sb.append(t)
        for ci in range(3):
            t = const.tile([P, 384], MMDT, tag=f"wv{ci}")
            nc.sync.dma_start(t[:], wv_d[ci])
            wv_sb.append(t)
        for cc in range(4):
            t = const.tile([P, 384], MMDT, tag=f"wp{cc}")
            nc.sync.dma_start(t[:], wp_d[cc])
            wp_sb.append(t)
        bq_sb = const.tile([P, 1], DT, tag="bq")
        nc.sync.dma_start(bq_sb[:], bq_d[:])
        bk_sb = const.tile([P, 1], DT, tag="bk")
        nc.sync.dma_start(bk_sb[:], bk_d[:])
        tri0_sb = const.tile([P, 256], BF, tag="tri0")
        nc.sync.dma_start(tri0_sb[:], tri0_d[:])
        bp_sb = const.tile([P, 384], DT, tag="bp")
        nc.sync.dma_start(bp_sb[:], bp_d[:])

        tri0_r = tri0_sb[:].rearrange("p (j r) -> p j r", r=128)

        # ---- per-batch-pair pipeline ---------------------------------------
        assert nb % 2 == 0

        def emit_proj(b, ao):
            for tcx in range(2):
                psy = psY.tile([P, 384], DT, tag="psY")
                for cc in range(4):
                    nc.tensor.matmul(
                        psy[:],
                        ao[:, 256 * cc + 128 * tcx : 256 * cc + 128 * tcx + 128],
                        wp_sb[cc][:],
                        start=(cc == 0),
                        stop=(cc == 3),
                    )
                ysb = yp.tile([P, 384], DT, tag="y")
                nc.vector.tensor_add(ysb[:], psy[:], bp_sb[:])
                nc.sync.dma_start(y_d[b, 128 * tcx : 128 * tcx + 128, :], ysb[:])

        pending_proj = None
        for bp_it in range((nb // 2) * repeat):
            b0 = 2 * (bp_it % (nb // 2))
            xt = []
            for ci in range(3):
                t = xtp.tile([P, 2 * T], MMDT, tag="xt")
                nc.sync.dma_start(
                    t[:].rearrange("p (b t) -> p b t", b=2),
                    xT[b0 : b0 + 2, 128 * ci : 128 * ci + 128, :].rearrange(
                        "b p t -> p b t"
                    ),
                )
                xt.append(t)

            # QK projection: 2 waves x 4 single-bank [128, 512] psum tiles
            # (bufs=2 so next chunk's matmuls overlap this chunk's bias-move)
            qk_sb = {}
            for w, name in ((0, "qt"), (1, "kt")):
                dst = qkp.tile([P, 2048], MMDT, tag=name)
                bias = bq_sb if w == 0 else bk_sb
                for oc in range(4):
                    ps = psA.tile([P, 512], DT, tag="psA")
                    for ci in range(3):
                        nc.tensor.matmul(
                            ps[:],
                            wqk_sb[ci][
                                :, 512 * w + 128 * oc : 512 * w + 128 * oc + 128
                            ],
                            xt[ci][:],
                            start=(ci == 0),
                            stop=(ci == 2),
                        )
                    dst_sl = dst[:, 512 * oc : 512 * oc + 512]
                    if (w + oc) % 2 == 0:
                        nc.vector.tensor_scalar_add(dst_sl, ps[:], bias[:, 0:1])
                    else:
                        nc.scalar.activation(
                            dst_sl, ps[:], AF.Identity, bias=bias[:, 0:1]
                        )
                qk_sb[name] = dst
            qt, kt = qk_sb["qt"], qk_sb["kt"]

            # V in token-major layout (per batch), ones column + eps pads
            v_sbs = []
            for bb in range(2):
                vt = vp.tile([P, 1024], BF, tag="v")
                nc.sync.dma_start(vt[:], vinit_d[:])
                v_sbs.append(vt)
            for tch in range(4):
                bb, tcx = tch // 2, tch % 2
                psv = psA.tile([P, 512], DT, tag="psA")
                for ci in range(3):
                    nc.tensor.matmul(
                        psv[:, 0:384],
                        xt[ci][:, 256 * bb + 128 * tcx : 256 * bb + 128 * tcx + 128],
                        wv_sb[ci][:],
                        start=(ci == 0),
                        stop=(ci == 2),
                    )
                half = v_sbs[bb][:, 512 * tcx : 512 * tcx + 512].rearrange(
                    "p (h c) -> p h c", c=HB
                )
                psv_r = psv[:, 0:384].rearrange("p (h c) -> p h c", c=48)
                nc.scalar.activation(half[:, :, 1:49], psv_r[:, :, :], AF.Copy)

            for bb in range(2):
                b = b0 + bb
                v_sb = v_sbs[bb]
                # S^T per head pair + exp + mask + PV, per half-batch psO
                pt = ptp.tile([P, 4096], BF, tag="pt")
                ao = aop.tile([P, 1024], MMDT, tag="ao")
                for h2 in range(2):
                    psoh = psO.tile([P, 512], DT, tag="psO")
                    for gg in range(2):
                        g = 2 * h2 + gg
                        if g == 1 and pending_proj is not None:
                            # flush the previous batch's deferred projection
                            # here so its matmuls queue behind this batch's
                            # first attention group (keeps PE stream dense)
                            emit_proj(*pending_proj)
                            pending_proj = None
                        pss = psS.tile([P, 1024], DT, tag="psS")
                        qb = 512 * g + 256 * bb
                        for j in range(2):
                            base = HB * j
                            # kx0: keys 0..127, all 256 queries
                            nc.tensor.matmul(
                                pss[:, 512 * j : 512 * j + 256],
                                kt[base : base + KA, qb : qb + 128],
                                qt[base : base + KA, qb : qb + 256],
                                start=True,
                                stop=True,
                            )
                            # kx1: keys 128..255, queries 128..255 only
                            # (q<128 is fully causal-masked for these keys)
                            nc.tensor.matmul(
                                pss[:, 512 * j + 256 : 512 * j + 384],
                                kt[base : base + KA, qb + 128 : qb + 256],
                                qt[base : base + KA, qb + 128 : qb + 256],
                                start=True,
                                stop=True,
                            )
                        pt_g = pt[:, 1024 * g : 1024 * g + 1024].rearrange(
                            "p (j r) -> p j r", r=512
                        )
                        pss_r = pss[:].rearrange("p (j r) -> p j r", r=512)
                        nc.scalar.activation(
                            pt_g[:, :, 0:384], pss_r[:, :, 0:384], AF.Exp
                        )
                        # triangular masks: diagonal blocks of kx0 (cols
                        # 0..127) and kx1 (cols 256..383) share the pattern
                        nc.vector.tensor_mul(
                            pt_g[:, :, 0:128], pt_g[:, :, 0:128], tri0_r
                        )
                        nc.vector.tensor_mul(
                            pt_g[:, :, 256:384], pt_g[:, :, 256:384], tri0_r
                        )
                        # PV for this group
                        for j in range(2):
                            h = 2 * g + j
                            pc = 1024 * g + 512 * j
                            nc.tensor.matmul(
                                psoh[HB * j : HB * j + HB, 256 * gg : 256 * gg + 256],
                                v_sb[:, HB * h : HB * h + HB],
                                pt[:, pc : pc + 256],
                                start=True,
                                stop=False,
                                tile_position=(0, HB * j),
                            )
                            nc.tensor.matmul(
                                psoh[
                                    HB * j : HB * j + HB,
                                    256 * gg + 128 : 256 * gg + 256,
                                ],
                                v_sb[:, 512 + HB * h : 512 + HB * h + HB],
                                pt[:, pc + 256 : pc + 384],
                                start=False,
                                stop=True,
                                tile_position=(0, HB * j),
                            )

                    # normalize this half-batch: rows 0 / 64 hold denominators
                    rec = aop.tile([1, 1024], DT, tag="rec")
                    if FAST_RECIP:
                        # custom-DVE op only works with partition-0 base on
                        # HW; stage the j1 den row (partition 64) to p0 via
                        # an ACT partition-crossing copy first
                        recs = aop.tile([1, 512], DT, tag="recs")
                        nc.scalar.activation(
                            recs[0:1, :], psoh[64:65, :], AF.Copy
                        )
                        nc.vector.reciprocal_approx_fast(
                            rec[0:1, 0:512], psoh[0:1, :]
                        )
                        nc.vector.reciprocal_approx_fast(
                            rec[0:1, 512:1024], recs[0:1, :]
                        )
                    else:
                        nc.vector.reciprocal(rec[0:1, 0:512], psoh[0:1, :])
                        nc.vector.reciprocal(rec[0:1, 512:1024], psoh[64:65, :])
                    denb = aop.tile([P, 512], DT, tag="denb")
                    nc.gpsimd.partition_broadcast(denb[:, :], rec[0:1, 512:1024])
                    nc.gpsimd.partition_broadcast(denb[0:64, :], rec[0:1, 0:512])
                    nc.vector.tensor_mul(
                        ao[:, 512 * h2 : 512 * h2 + 512], psoh[:], denb[:]
                    )

                # output projection: deferred — emitted during the next
                # batch's attention so PE never waits on the normalize tail
                pending_proj = (b, ao)

        if pending_proj is not None:
            emit_proj(*pending_proj)
            pending_proj = None

        if timing:
            nc.sync.dma_start(ydum_d[:], bp_sb[:, 0:4])

    nc.compile()
    return nc


def make_consts(attn_w, attn_b, proj_w, proj_b):
    attn_w = np.asarray(attn_w, dtype=np.float32)
    attn_b = np.asarray(attn_b, dtype=np.float32)
    proj_w = np.asarray(proj_w, dtype=np.float32)
    proj_b = np.asarray(proj_b, dtype=np.float32)

    s = 1.0 / np.sqrt(HD)
    Wq, Wk, Wv = attn_w[0:C], attn_w[C : 2 * C], attn_w[2 * C : 3 * C]
    bq, bk, bv = attn_b[0:C], attn_b[C : 2 * C], attn_b[2 * C : 3 * C]

    # WQK: [C, 1024] -> [3, 128, 1024]
    M = np.zeros((C, 1024), dtype=np.float32)
    for h in range(NH):
        Wq_h = Wq[HD * h : HD * h + HD]  # [48, C]
        Wk_h = Wk[HD * h : HD * h + HD]
        bq_h = bq[HD * h : HD * h + HD]
        bk_h = bk[HD * h : HD * h + HD]
        # q-hat block
        M[:, HB * h : HB * h + HD] = (s * Wq_h).T
        M[:, HB * h + 48] = s * (bk_h @ Wq_h)  # c_q row
        # (row 49 of q-hat is the ones row via bias)
        # k-hat block
        M[:, 512 + HB * h : 512 + HB * h + HD] = Wk_h.T
        # (row 48 of k-hat is the ones row via bias)
        M[:, 512 + HB * h + 49] = s * (bq_h @ Wk_h)  # c_k row
    WQK = np.ascontiguousarray(M.reshape(C, 1024).reshape(3, P, 1024))

    # WV: [C, 384] -> [3, 128, 384]; col HD*h+j = Wv row HD*h+j (packed)
    V = np.zeros((C, 384), dtype=np.float32)
    for h in range(NH):
        V[:, HD * h : HD * h + HD] = Wv[HD * h : HD * h + HD].T
    WV = np.ascontiguousarray(V.reshape(3, P, 384))

    # WP: [512, 384] -> [4, 128, 384]; row HB*h + 1 + j = proj_w[:, HD*h+j]
    Wp_aug = np.zeros((512, C), dtype=np.float32)
    for h in range(NH):
        Wp_aug[HB * h + 1 : HB * h + 1 + HD, :] = proj_w[:, HD * h : HD * h + HD].T
    WP = np.ascontiguousarray(Wp_aug.reshape(4, P, 384))

    BQ = np.zeros((P, 1), dtype=np.float32)
    BQ[49, 0] = 1.0
    BQ[49 + HB, 0] = 1.0
    BK = np.zeros((P, 1), dtype=np.float32)
    BK[48, 0] = 1.0
    BK[48 + HB, 0] = 1.0

    # causal 0/1 mask for S^T[k, q] diagonal tiles (repeated x2 for head pairs)
    kk = np.arange(128)[:, None]
    qq = np.arange(128)[None, :]
    tri = (qq >= kk).astype(np.float32)  # [128k, 128q]
    TRI0 = np.ascontiguousarray(np.concatenate([tri, tri], axis=1)).astype(BF16_NP)

    bp_eff = proj_b + proj_w @ bv
    BP = np.ascontiguousarray(np.broadcast_to(bp_eff[None, :], (P, 384))).astype(
        np.float32
    )

    # v-init pattern: ones column at 64h, EPS_PAD at cols 49..63 of each block
    vinit_row = np.zeros(1024, dtype=np.float32)
    for kx in range(2):
        for h in range(NH):
            off = 512 * kx + HB * h
            vinit_row[off] = 1.0
            vinit_row[off + 49 : off + HB] = EPS_PAD
    VINIT = np.ascontiguousarray(np.broadcast_to(vinit_row[None, :], (P, 1024))).astype(
        BF16_NP
    )

    mmnp = BF16_NP if MM_BF16 else np.float32
    WQK = WQK.astype(mmnp)
    WV = WV.astype(mmnp)
    WP = WP.astype(mmnp)

    return {
        "vinit": VINIT,
        "wqk": WQK,
        "wv": WV,
        "wp": WP,
        "bq": BQ,
        "bk": BK,
        "tri0": TRI0,
        "bp": BP,
    }


_NC_CACHE = {}


def get_nc(nb: int = NB):
    if nb not in _NC_CACHE:
        _NC_CACHE[nb] = build_nc(nb)
    return _NC_CACHE[nb]


def make_in_maps(x, attn_w, attn_b, proj_w, proj_b):
    x = np.asarray(x, dtype=np.float32)
    consts = make_consts(attn_w, attn_b, proj_w, proj_b)
    in_maps = []
    for core in range(N_CORES):
        xs = x[core * NB : (core + 1) * NB]  # [NB, T, C]
        xTl = np.ascontiguousarray(xs.transpose(0, 2, 1))  # [NB, C, T]
        if MM_BF16:
            xTl = xTl.astype(BF16_NP)
        m = {"xT": xTl}
        m.update(consts)
        in_maps.append(m)
    return in_maps


def kernel(x, attn_w, attn_b, proj_w, proj_b):
    nc = get_nc(NB)
    in_maps = make_in_maps(x, attn_w, attn_b, proj_w, proj_b)
    res = run_bass_kernel_spmd(nc, in_maps, core_ids=list(range(N_CORES)))
    out = np.concatenate(
        [res.results[i]["y"] for i in range(N_CORES)], axis=0
    ).astype(np.float32)
    return out


# revision 21
# speedup vs baseline: 3.0669x; 1.0120x over previous
"""Causal self-attention Trainium2 kernel (B=256, T=256, C=384, 8 heads x 48).

Strategy: pure data-parallel over batch across 8 NeuronCores (32 batches per
core, no collectives). All layouts are arranged on the host so the device
kernel never transposes anything:

  - x is sent transposed per batch: xT [nb, C, T].
  - QK projection computes q^T / k^T in "feature-major" layout [o', tokens]
    with heads padded to 64-row blocks, augmented with 2 extra contraction
    rows that carry the q/k bias cross terms, so scores come out exactly
    (up to a per-head constant, which softmax ignores).  K_contract = 50.
  - Scores are computed transposed, S^T[k, q], per head with 2-head row-tiled
    matmul concurrency (head dim 50 <= 64 rows).  Causal block-sparsity: the
    (k in 128..255, q in 0..127) quadrant is fully masked, so its matmul,
    exp, mask and PV contributions are skipped entirely.  Per j-half layout:
    [kx0 q0..255 | kx1 q128..255 | dead].
  - Softmax skips the max-subtraction (inputs are well-scaled gaussians),
    exp on ACT straight PSUM->SBUF (strided, skipping the dead region),
    causal mask applied multiplicatively to the two triangular 128-blocks
    (both use the same tri pattern) on DVE.
  - The PV matmul consumes V in token-major layout (computed directly by
    swapping stationary/moving operands - no transpose), augmented with a
    leading ones column per head so row 0 of each head block is the softmax
    denominator.  Col-tiled 2-head concurrency; output per half-batch into a
    single-bank [128, 512] PSUM tile (bufs=2) for pipelining.
  - Normalization per half-batch: strided 2-row reciprocal_approx_fast of
    the denominator rows (~5x faster than vector.reciprocal), GpSimd
    partition-broadcast, one fused multiply PSUM->SBUF.
  - Output projection consumes the attention output directly in its
    [c', token] layout; V-bias is folded into the projection bias on host.

Matmuls run in bf16 (fp32 PSUM accumulation).
"""

import os
import sys

import numpy as np

try:
    import ml_dtypes

    BF16_NP = ml_dtypes.bfloat16
except ImportError:  # pragma: no cover
    BF16_NP = None

for _p in ("/opt/trn_rl_repo",):
    if os.path.isdir(_p) and _p not in sys.path:
        sys.path.insert(0, _p)

from contextlib import ExitStack

import concourse.bass as bass
import concourse.bacc as bacc
import concourse.tile as tile
from concourse import mybir
from concourse.bass_utils import run_bass_kernel_spmd

P = 128
T = 256
C = 384
NH = 8
HD = 48
KA = 50  # augmented contraction rows per head (48 + cq/ck row + ones row)
HB = 64  # padded head block stride
DT = mybir.dt.float32
DTR = mybir.dt.float32r
BF = mybir.dt.bfloat16
AF = mybir.ActivationFunctionType
N_CORES = 8
B_FULL = 256
NB = B_FULL // N_CORES  # batches per core

EPS_PAD = 1e-20  # value for padded V columns (keeps reciprocal finite)

MM_BF16 = os.environ.get("KERNEL_MM_BF16", "1") == "1"
MMDT = BF if MM_BF16 else DTR
FAST_RECIP = os.environ.get("KERNEL_FAST_RECIP", "1") == "1"


def build_nc(nb: int = NB, debug: bool = False, repeat: int = 1, timing: bool = False):
    nc = bacc.Bacc(None)

    xT = nc.declare_dram_parameter("xT", [nb, C, T], MMDT, isOutput=False)
    wqk_d = nc.declare_dram_parameter("wqk", [3, P, 1024], MMDT, isOutput=False)
    wv_d = nc.declare_dram_parameter("wv", [3, P, 384], MMDT, isOutput=False)
    wp_d = nc.declare_dram_parameter("wp", [4, P, 384], MMDT, isOutput=False)
    bq_d = nc.declare_dram_parameter("bq", [P, 1], DT, isOutput=False)
    bk_d = nc.declare_dram_parameter("bk", [P, 1], DT, isOutput=False)
    tri0_d = nc.declare_dram_parameter("tri0", [P, 512], BF, isOutput=False)
    bp_d = nc.declare_dram_parameter("bp", [P, 384], DT, isOutput=False)
    vinit_d = nc.declare_dram_parameter("vinit", [P, 1024], BF, isOutput=False)
    if timing:
        y_d = nc.dram_tensor("y_int", [nb, T, C], DT)
        ydum_d = nc.declare_dram_parameter("ydum", [P, 4], DT, isOutput=True)
    else:
        y_d = nc.declare_dram_parameter("y", [nb, T, C], DT, isOutput=True)

    with tile.TileContext(nc) as tc, ExitStack() as ctx:
        const = ctx.enter_context(tc.tile_pool(name="const", bufs=1))
        xtp = ctx.enter_context(tc.tile_pool(name="xt", bufs=9))
        qkp = ctx.enter_context(tc.tile_pool(name="qkt", bufs=2))
        vp = ctx.enter_context(tc.tile_pool(name="v", bufs=3))
        ptp = ctx.enter_context(tc.tile_pool(name="pt", bufs=2))
        aop = ctx.enter_context(tc.tile_pool(name="ao", bufs=3))
        yp = ctx.enter_context(tc.tile_pool(name="y", bufs=6))
        psA = ctx.enter_context(
            tc.tile_pool(name="psA", bufs=2, space=bass.MemorySpace.PSUM)
        )
        psS = ctx.enter_context(
            tc.tile_pool(name="psS", bufs=1, space=bass.MemorySpace.PSUM)
        )
        psO = ctx.enter_context(
            tc.tile_pool(name="psO", bufs=2, space=bass.MemorySpace.PSUM)
        )
        psY = ctx.enter_context(
            tc.tile_pool(name="psY", bufs=2, space=bass.MemorySpace.PSUM)
        )

        # ---- load constants ------------------------------------------------
        wqk_sb = []
        wv_sb = []
        wp_sb = []
        for ci in range(3):
            t = const.tile([P, 1024], MMDT, tag=f"wqk{ci}")
            nc.sync.dma_start(t[:], wqk_d[ci])
            wqk_sb.append(t)
        for ci in range(3):
            t = const.tile([P, 384], MMDT, tag=f"wv{ci}")
            nc.sync.dma_start(t[:], wv_d[ci])
            wv_sb.append(t)
        for cc in range(4):
            t = const.tile([P, 384], MMDT, tag=f"wp{cc}")
            nc.sync.dma_start(t[:], wp_d[cc])
            wp_sb.append(t)
        bq_sb = const.tile([P, 1], DT, tag="bq")
        nc.sync.dma_start(bq_sb[:], bq_d[:])
        bk_sb = const.tile([P, 1], DT, tag="bk")
        nc.sync.dma_start(bk_sb[:], bk_d[:])
        tri0_sb = const.tile([P, 512], BF, tag="tri0")
        nc.sync.dma_start(tri0_sb[:], tri0_d[:])
        bp_sb = const.tile([P, 384], DT, tag="bp")
        nc.sync.dma_start(bp_sb[:], bp_d[:])

        tri4_r = tri0_sb[:].rearrange("p (j b r) -> p j b r", j=2, b=2, r=128)

        # ---- per-batch-pair pipeline ---------------------------------------
        assert nb % 2 == 0

        def emit_proj(b, ao):
            for tcx in range(2):
                psy = psY.tile([P, 384], DT, tag="psY")
                for cc in range(4):
                    nc.tensor.matmul(
                        psy[:],
                        ao[:, 256 * cc + 128 * tcx : 256 * cc + 128 * tcx + 128],
                        wp_sb[cc][:],
                        start=(cc == 0),
                        stop=(cc == 3),
                    )
                ysb = yp.tile([P, 384], DT, tag="y")
                nc.vector.tensor_add(ysb[:], psy[:], bp_sb[:])
                nc.sync.dma_start(y_d[b, 128 * tcx : 128 * tcx + 128, :], ysb[:])

        pending_proj = None
        for bp_it in range((nb // 2) * repeat):
            b0 = 2 * (bp_it % (nb // 2))
            xt = []
            for ci in range(3):
                t = xtp.tile([P, 2 * T], MMDT, tag="xt")
                nc.sync.dma_start(
                    t[:].rearrange("p (b t) -> p b t", b=2),
                    xT[b0 : b0 + 2, 128 * ci : 128 * ci + 128, :].rearrange(
                        "b p t -> p b t"
                    ),
                )
                xt.append(t)

            # QK projection: 2 waves x 4 single-bank [128, 512] psum tiles
            # (bufs=2 so next chunk's matmuls overlap this chunk's bias-move)
            qk_sb = {}
            for w, name in ((0, "qt"), (1, "kt")):
                dst = qkp.tile([P, 2048], MMDT, tag=name)
                bias = bq_sb if w == 0 else bk_sb
                for oc in range(4):
                    ps = psA.tile([P, 512], DT, tag="psA")
                    for ci in range(3):
                        nc.tensor.matmul(
                            ps[:],
                            wqk_sb[ci][
                                :, 512 * w + 128 * oc : 512 * w + 128 * oc + 128
                            ],
                            xt[ci][:],
                            start=(ci == 0),
                            stop=(ci == 2),
                        )
                    dst_sl = dst[:, 512 * oc : 512 * oc + 512]
                    if (w + oc) % 2 == 0:
                        nc.vector.tensor_scalar_add(dst_sl, ps[:], bias[:, 0:1])
                    else:
                        nc.scalar.activation(
                            dst_sl, ps[:], AF.Identity, bias=bias[:, 0:1]
                        )
                qk_sb[name] = dst
            qt, kt = qk_sb["qt"], qk_sb["kt"]

            # V in token-major layout (per batch), ones column + eps pads
            v_sbs = []
            for bb in range(2):
                vt = vp.tile([P, 1024], BF, tag="v")
                nc.sync.dma_start(vt[:], vinit_d[:])
                v_sbs.append(vt)
            for tch in range(4):
                bb, tcx = tch // 2, tch % 2
                psv = psA.tile([P, 512], DT, tag="psA")
                for ci in range(3):
                    nc.tensor.matmul(
                        psv[:, 0:384],
                        xt[ci][:, 256 * bb + 128 * tcx : 256 * bb + 128 * tcx + 128],
                        wv_sb[ci][:],
                        start=(ci == 0),
                        stop=(ci == 2),
                    )
                half = v_sbs[bb][:, 512 * tcx : 512 * tcx + 512].rearrange(
                    "p (h c) -> p h c", c=HB
                )
                psv_r = psv[:, 0:384].rearrange("p (h c) -> p h c", c=48)
                nc.scalar.activation(half[:, :, 1:49], psv_r[:, :, :], AF.Copy)

            for bb in range(2):
                b = b0 + bb
                v_sb = v_sbs[bb]
                # S^T per head pair + exp + mask + PV, per half-batch psO
                pt = ptp.tile([P, 4096], BF, tag="pt")
                ao = aop.tile([P, 1024], MMDT, tag="ao")
                for h2 in range(2):
                    psoh = psO.tile([P, 512], DT, tag="psO")
                    for gg in range(2):
                        g = 2 * h2 + gg
                        if g == 1 and pending_proj is not None:
                            # flush the previous batch's deferred projection
                            # here so its matmuls queue behind this batch's
                            # first attention group (keeps PE stream dense)
                            emit_proj(*pending_proj)
                            pending_proj = None
                        pss = psS.tile([P, 1024], DT, tag="psS")
                        qb = 512 * g + 256 * bb
                        for j in range(2):
                            base = HB * j
                            # kx0: keys 0..127, all 256 queries
                            nc.tensor.matmul(
                                pss[:, 512 * j : 512 * j + 256],
                                kt[base : base + KA, qb : qb + 128],
                                qt[base : base + KA, qb : qb + 256],
                                start=True,
                                stop=True,
                            )
                            # kx1: keys 128..255, queries 128..255 only
                            # (q<128 is fully causal-masked for these keys)
                            nc.tensor.matmul(
                                pss[:, 512 * j + 256 : 512 * j + 384],
                                kt[base : base + KA, qb + 128 : qb + 256],
                                qt[base : base + KA, qb + 128 : qb + 256],
                                start=True,
                                stop=True,
                            )
                        pt_g = pt[:, 1024 * g : 1024 * g + 1024].rearrange(
                            "p (j r) -> p j r", r=512
                        )
                        pss_r = pss[:].rearrange("p (j r) -> p j r", r=512)
                        nc.scalar.activation(
                            pt_g[:, :, 0:384], pss_r[:, :, 0:384], AF.Exp
                        )
                        # triangular masks: diagonal blocks of kx0 (cols
                        # 0..127) and kx1 (cols 256..383) share the pattern;
                        # one strided mul covers both via a stride-2 mid dim
                        pg4 = pt[:, 1024 * g : 1024 * g + 1024].rearrange(
                            "p (j b r) -> p j b r", j=2, b=4, r=128
                        )
                        msl = pg4[:, :, 0:3:2, :]
                        nc.vector.tensor_mul(msl, msl, tri4_r)
                        # PV for this group
                        for j in range(2):
                            h = 2 * g + j
                            pc = 1024 * g + 512 * j
                            nc.tensor.matmul(
                                psoh[HB * j : HB * j + HB, 256 * gg : 256 * gg + 256],
                                v_sb[:, HB * h : HB * h + HB],
                                pt[:, pc : pc + 256],
                                start=True,
                                stop=False,
                                tile_position=(0, HB * j),
                            )
                            nc.tensor.matmul(
                                psoh[
                                    HB * j : HB * j + HB,
                                    256 * gg + 128 : 256 * gg + 256,
                                ],
                                v_sb[:, 512 + HB * h : 512 + HB * h + HB],
                                pt[:, pc + 256 : pc + 384],
                                start=False,
                                stop=True,
                                tile_position=(0, HB * j),
                            )

                    # normalize this half-batch: rows 0 / 64 hold denominators
                    rec = aop.tile([1, 1024], DT, tag="rec")
                    if FAST_RECIP:
                        # custom-DVE op only works with partition-0 base on
                        # HW; stage the j1 den row (partition 64) to p0 via
                        # an ACT partition-crossing copy first
                        recs = aop.tile([1, 512], DT, tag="recs")
                        nc.scalar.activation(
                            recs[0:1, :], psoh[64:65, :], AF.Copy
                        )
                        nc.vector.reciprocal_approx_fast(
                            rec[0:1, 0:512], psoh[0:1, :]
                        )
                        nc.vector.reciprocal_approx_fast(
                            rec[0:1, 512:1024], recs[0:1, :]
                        )
                    else:
                        nc.vector.reciprocal(rec[0:1, 0:512], psoh[0:1, :])
                        nc.vector.reciprocal(rec[0:1, 512:1024], psoh[64:65, :])
                    denb = aop.tile([P, 512], DT, tag="denb")
                    nc.gpsimd.partition_broadcast(denb[:, :], rec[0:1, 512:1024])
                    nc.gpsimd.partition_broadcast(denb[0:64, :], rec[0:1, 0:512])
                    nc.vector.tensor_mul(
                        ao[:, 512 * h2 : 512 * h2 + 512], psoh[:], denb[:]
                    )

                # output projection: deferred — emitted during the next
                # batch's attention so PE never waits on the normalize tail
                pending_proj = (b, ao)

        if pending_proj is not None:
            emit_proj(*pending_proj)
            pending_proj = None

        if timing:
            nc.sync.dma_start(ydum_d[:], bp_sb[:, 0:4])

    nc.compile()
    return nc


def make_consts(attn_w, attn_b, proj_w, proj_b):
    attn_w = np.asarray(attn_w, dtype=np.float32)
    attn_b = np.asarray(attn_b, dtype=np.float32)
    proj_w = np.asarray(proj_w, dtype=np.float32)
    proj_b = np.asarray(proj_b, dtype=np.float32)

    s = 1.0 / np.sqrt(HD)
    Wq, Wk, Wv = attn_w[0:C], attn_w[C : 2 * C], attn_w[2 * C : 3 * C]
    bq, bk, bv = attn_b[0:C], attn_b[C : 2 * C], attn_b[2 * C : 3 * C]

    # WQK: [C, 1024] -> [3, 128, 1024]
    M = np.zeros((C, 1024), dtype=np.float32)
    for h in range(NH):
        Wq_h = Wq[HD * h : HD * h + HD]  # [48, C]
        Wk_h = Wk[HD * h : HD * h + HD]
        bq_h = bq[HD * h : HD * h + HD]
        bk_h = bk[HD * h : HD * h + HD]
        # q-hat block
        M[:, HB * h : HB * h + HD] = (s * Wq_h).T
        M[:, HB * h + 48] = s * (bk_h @ Wq_h)  # c_q row
        # (row 49 of q-hat is the ones row via bias)
        # k-hat block
        M[:, 512 + HB * h : 512 + HB * h + HD] = Wk_h.T
        # (row 48 of k-hat is the ones row via bias)
        M[:, 512 + HB * h + 49] = s * (bq_h @ Wk_h)  # c_k row
    WQK = np.ascontiguousarray(M.reshape(C, 1024).reshape(3, P, 1024))

    # WV: [C, 384] -> [3, 128, 384]; col HD*h+j = Wv row HD*h+j (packed)
    V = np.zeros((C, 384), dtype=np.float32)
    for h in range(NH):
        V[:, HD * h : HD * h + HD] = Wv[HD * h : HD * h + HD].T
    WV = np.ascontiguousarray(V.reshape(3, P, 384))

    # WP: [512, 384] -> [4, 128, 384]; row HB*h + 1 + j = proj_w[:, HD*h+j]
    Wp_aug = np.zeros((512, C), dtype=np.float32)
    for h in range(NH):
        Wp_aug[HB * h + 1 : HB * h + 1 + HD, :] = proj_w[:, HD * h : HD * h + HD].T
    WP = np.ascontiguousarray(Wp_aug.reshape(4, P, 384))

    BQ = np.zeros((P, 1), dtype=np.float32)
    BQ[49, 0] = 1.0
    BQ[49 + HB, 0] = 1.0
    BK = np.zeros((P, 1), dtype=np.float32)
    BK[48, 0] = 1.0
    BK[48 + HB, 0] = 1.0

    # causal 0/1 mask for S^T[k, q] diagonal tiles (4 reps: j-pairs x kx0/kx1)
    kk = np.arange(128)[:, None]
    qq = np.arange(128)[None, :]
    tri = (qq >= kk).astype(np.float32)  # [128k, 128q]
    TRI0 = np.ascontiguousarray(np.concatenate([tri] * 4, axis=1)).astype(BF16_NP)

    bp_eff = proj_b + proj_w @ bv
    BP = np.ascontiguousarray(np.broadcast_to(bp_eff[None, :], (P, 384))).astype(
        np.float32
    )

    # v-init pattern: ones column at 64h, EPS_PAD at cols 49..63 of each block
    vinit_row = np.zeros(1024, dtype=np.float32)
    for kx in range(2):
        for h in range(NH):
            off = 512 * kx + HB * h
            vinit_row[off] = 1.0
            vinit_row[off + 49 : off + HB] = EPS_PAD
    VINIT = np.ascontiguousarray(np.broadcast_to(vinit_row[None, :], (P, 1024))).astype(
        BF16_NP
    )

    mmnp = BF16_NP if MM_BF16 else np.float32
    WQK = WQK.astype(mmnp)
    WV = WV.astype(mmnp)
    WP = WP.astype(mmnp)

    return {
        "vinit": VINIT,
        "wqk": WQK,
        "wv": WV,
        "wp": WP,
        "bq": BQ,
        "bk": BK,
        "tri0": TRI0,
        "bp": BP,
    }


_NC_CACHE = {}


def get_nc(nb: int = NB):
    if nb not in _NC_CACHE:
        _NC_CACHE[nb] = build_nc(nb)
    return _NC_CACHE[nb]


def make_in_maps(x, attn_w, attn_b, proj_w, proj_b):
    x = np.asarray(x, dtype=np.float32)
    consts = make_consts(attn_w, attn_b, proj_w, proj_b)
    in_maps = []
    for core in range(N_CORES):
        xs = x[core * NB : (core + 1) * NB]  # [NB, T, C]
        xTl = np.ascontiguousarray(xs.transpose(0, 2, 1))  # [NB, C, T]
        if MM_BF16:
            xTl = xTl.astype(BF16_NP)
        m = {"xT": xTl}
        m.update(consts)
        in_maps.append(m)
    return in_maps


def kernel(x, attn_w, attn_b, proj_w, proj_b):
    nc = get_nc(NB)
    in_maps = make_in_maps(x, attn_w, attn_b, proj_w, proj_b)
    res = run_bass_kernel_spmd(nc, in_maps, core_ids=list(range(N_CORES)))
    out = np.concatenate(
        [res.results[i]["y"] for i in range(N_CORES)], axis=0
    ).astype(np.float32)
    return out
